# revision 1
# baseline (speedup 1.0000x reference)
"""Zero-communication Trainium2 attention-head kernel (softmax over queries).

Every core computes the FULL softmax denominator (all scores + exp) locally
and the output rows of its own query slab only.  No cross-core traffic, so
each core's NEFF span is independent of launch stagger — the max-over-cores
exec time collapses to per-core compute.

Per core (inputs host-prepped bf16 hi/lo; q columns ROTATED so the core's
slab occupies i in [0, SLAB)):
  1. Stream + project q (3-term bf16, result duplicated into both PSUM
     halves via tile_position), split to stacked Q1=[qh;ql], Q2=[ql;qh].
     Same for k -> KS=[kh;kl].  v projected per j-chunk -> vt bf16.
  2. Per j-tile (128 keys): 8 score chunks [128, ICH] via 2 stacked
     matmuls each (all 4 hi/lo cross terms: KS^T Q1 = kh qh + kl ql,
     KS^T Q2 = kh ql + kl qh).  Chunks processed in 4 pairs; one DVE
     tensor_tensor_reduce per pair yields the negated pair-max (a valid
     exp bias: >= chunkmax-80, <= columnmax).  ACT exps every chunk with
     its pair bias, accumulating row sums; the slab chunk's exp values
     are kept in bf16.  Small per-tile combine -> colsum; 1/colsum and
     the slab pair's rescale weight are folded into vt.
  3. attn: ps_o[128, SLAB/2] (i halves stacked on partitions) accumulates
     vt-row x slab-exp matmuls over all NT tiles.
"""

import numpy as np

C = 8
QK = 64
VD = 64


def build_nc2(seq=8192, d=1024, reps=1, warmup=True):
    import concourse.bacc as bacc
    import concourse.mybir as mybir

    f32 = mybir.dt.float32
    bf16 = mybir.dt.bfloat16
    AX = mybir.AxisListType.X
    ALU = mybir.AluOpType
    ACTF = mybir.ActivationFunctionType

    NDT = d // 128
    SLAB = seq // C
    ICH = SLAB                   # score chunk width (i cols)
    NT = seq // 128              # j tiles
    QCH = 512                    # q stream/proj chunk (i cols)
    NQC = seq // QCH
    KCH = 512                    # k/v stream chunk (j cols)
    NKC = seq // KCH
    TPK = KCH // 128             # j tiles per k/v chunk
    QUN = NDT * QCH              # arena unit cols (one of hi/lo of a chunk)
    NDMA = NQC + NKC             # chunked input DMAs per iteration
    # chunk processed at position p: order 1..7 then 0 (slab last)
    POSC = [1, 2, 3, 4, 5, 6, 7, 0]
    INF = 3.0e38

    nc = bacc.Bacc(target_bir_lowering=False, debug=False)

    def din(name, w):
        return nc.declare_dram_parameter(name, [128, w], bf16, isOutput=False)

    q_d = din("qin", NQC * 2 * QUN)    # per chunk: [hi unit | lo unit]
    kv_d = din("kvin", NKC * 3 * QUN)  # per chunk: [kh | kl | v]
    wqh_d, wql_d = din("wqh", NDT * QK), din("wql", NDT * QK)
    wkh_d, wkl_d = din("wkh", NDT * QK), din("wkl", NDT * QK)
    wv_d = din("wv", NDT * VD)
    out_d = nc.declare_dram_parameter("out", [128, SLAB // 2], f32, isOutput=True)

    from contextlib import ExitStack

    with ExitStack() as ctx:
        block = ctx.enter_context(nc.Block())
        sem = lambda n: ctx.enter_context(nc.semaphore(n))
        sb = lambda n, shape, dt: ctx.enter_context(nc.sbuf_tensor(n, shape, dt))
        ps = lambda n, shape: ctx.enter_context(nc.psum_tensor(n, shape, f32))

        s_w = sem("s_w")          # 80/iter
        s_in = sem("s_in")        # NUNIT*16/iter
        s_qproj = sem("s_qproj")  # NQC/iter
        s_kproj = sem("s_kproj")  # NKC/iter
        s_vproj = sem("s_vproj")  # NKC/iter
        s_qsplit = sem("s_qsplit")  # NQC/iter
        s_ksplit = sem("s_ksplit")  # NKC/iter
        s_vtc = sem("s_vtc")      # NKC/iter
        s_sc = sem("s_sc")        # 8*NT/iter
        s_mx = sem("s_mx")        # 4*NT/iter
        s_ex = sem("s_ex")        # 8*NT/iter
        s_d = sem("s_d")          # NT/iter
        s_e2 = sem("s_e2")        # NT/iter
        s_vt2 = sem("s_vt2")      # NT/iter
        s_attn = sem("s_attn")    # NT/iter
        s_oc = sem("s_oc")        # 1/iter
        s_out = sem("s_out")      # 16/iter
        s_qcp = sem("s_qcp")      # 2*NQC/iter (q split copies)
        s_kcp = sem("s_kcp")      # 2*NKC/iter (k split copies)
        s_cmb = sem("s_cmb")      # 6*NT/iter (combine chain)

        arena_q = sb("arena_q", [128, 2 * 2 * QUN], bf16)
        arena_kv = sb("arena_kv", [128, 2 * 3 * QUN], bf16)
        Q1 = sb("Q1", [128, seq], bf16)
        Q2 = sb("Q2", [128, seq], bf16)
        KS = sb("KS", [128, seq], bf16)
        ktmp = sb("ktmp", [128, KCH], bf16)
        vt = sb("vt", [128, NT * VD], bf16)
        wqh = sb("wqh_s", [128, NDT * QK], bf16)
        wql = sb("wql_s", [128, NDT * QK], bf16)
        wkh = sb("wkh_s", [128, NDT * QK], bf16)
        wkl = sb("wkl_s", [128, NDT * QK], bf16)
        wv = sb("wv_s", [128, NDT * VD], bf16)
        slab_e = sb("slab_e", [128, 2 * ICH], bf16)
        scr = sb("scr", [128, 2 * ICH], bf16)  # ACT non-slab exp dump (x2)
        nmax = sb("nmax", [128, 5 * NT], f32)
        ssum = sb("ssum", [128, 8 * NT], f32)
        nB = sb("nB", [128, NT], f32)
        e_all = sb("e_all", [128, 5 * NT], f32)
        spair = sb("spair", [128, 3 * NT], f32)
        sw_all = sb("sw_all", [128, 3 * NT], f32)
        sw2 = sb("sw2", [128, 2 * NT], f32)
        cs2 = sb("cs2", [128, NT], f32)
        cs_all = sb("cs_all", [128, NT], f32)
        rS_all = sb("rS_all", [128, NT], f32)
        fS_all = sb("fS_all", [128, NT], f32)
        out_sb = sb("out_sb", [128, SLAB // 2], f32)

        ps_ab = ps("ps_ab", [128, max(2 * ICH, 4 * QCH)])
        ps_c = ps("ps_c", [128, max(ICH, 2 * QCH)])
        ps_o = ps("ps_o", [128, SLAB // 2])
        ps_v = ps("ps_v", [128, 512])

        def proj_slot(r):
            if r % 6 < 4:
                off = (r % 4) * QCH
                return ps_ab[:, off : off + QCH]
            off = (r % 2) * QCH
            return ps_c[:, off : off + QCH]

        def pos_slot(p):
            # positions 0..6 alternate ps_ab halves; 7 -> ps_c
            if p == 7:
                return ps_c[:, 0:ICH]
            return ps_ab[:, (p % 2) * ICH : (p % 2 + 1) * ICH]

        # ---------------- SYNC: DMAs (one dma per chunk, serialized) -----
        @block.sync
        def _(s):
          for it in range(reps):
            if it > 0:
                s.wait_ge(s_vproj, it * NKC)  # arenas fully consumed

            def dma(dst, src, sem_=s_in):
                s.dma_start(out=dst, in_=src).then_inc(sem_, 16)

            dma(wqh[:, :], wqh_d[:, :], s_w)
            dma(wql[:, :], wql_d[:, :], s_w)
            dma(wkh[:, :], wkh_d[:, :], s_w)
            dma(wkl[:, :], wkl_d[:, :], s_w)
            dma(wv[:, :], wv_d[:, :], s_w)

            n = 0
            for ic in range(NQC):
                if ic >= 2:
                    s.wait_ge(s_qproj, it * NQC + ic - 1)  # slot free
                dma(arena_q[:, (ic % 2) * 2 * QUN : (ic % 2 + 1) * 2 * QUN],
                    q_d[:, ic * 2 * QUN : (ic + 1) * 2 * QUN])
                n += 1
                # serialize so every s_in level is an ordered sync point
                s.wait_ge(s_in, it * NDMA * 16 + n * 16)
            for jc in range(NKC):
                if jc >= 2:
                    s.wait_ge(s_vproj, it * NKC + jc - 1)
                dma(arena_kv[:, (jc % 2) * 3 * QUN : (jc % 2 + 1) * 3 * QUN],
                    kv_d[:, jc * 3 * QUN : (jc + 1) * 3 * QUN])
                n += 1
                s.wait_ge(s_in, it * NDMA * 16 + n * 16)

            s.wait_ge(s_oc, it + 1)
            s.dma_start(out=out_d[:, :], in_=out_sb[:, :]).then_inc(s_out, 16)
            s.wait_ge(s_out, it * 16 + 16)

        # ---------------- TENSOR (PE) ----------------
        @block.tensor
        def _(t):
          for it in range(reps):
            for w_ in range(40 if (warmup and it == 0) else 0):
                t.matmul(
                    ps_ab[0:64, 0:QCH], Q1[:, 0:64], Q1[:, 512 : 512 + QCH],
                    start=(w_ == 0), stop=False,
                )
            t.wait_ge(s_w, it * 80 + 80)
            # q projection
            for ic in range(NQC):
                r = ic
                t.wait_ge(s_in, it * NDMA * 16 + (ic + 1) * 16)
                if r >= 6:
                    t.wait_ge(s_qsplit, it * NQC + (r - 6) + 1)
                sl = proj_slot(r)
                qb = (ic % 2) * 2 * QUN
                uh = arena_q[:, qb : qb + QUN]
                ul = arena_q[:, qb + QUN : qb + 2 * QUN]
                for pos in (0, 64):
                    for dd in range(NDT):
                        terms = ((wqh, uh), (wqh, ul), (wql, uh))
                        for ti, (W, X) in enumerate(terms):
                            mm = t.matmul(
                                sl[pos : pos + 64, :],
                                W[:, dd * QK : (dd + 1) * QK],
                                X[:, dd * QCH : (dd + 1) * QCH],
                                start=(dd == 0 and ti == 0),
                                stop=(dd == NDT - 1 and ti == 2),
                                tile_position=(0, pos),
                            )
                mm.then_inc(s_qproj, 1)
            # k + v projection
            for jc in range(NKC):
                r = NQC + jc
                t.wait_ge(s_in, it * NDMA * 16 + (NQC + jc + 1) * 16)
                pidx = r - 6
                if pidx >= 0:
                    if pidx < NQC:
                        t.wait_ge(s_qsplit, it * NQC + pidx + 1)
                    else:
                        t.wait_ge(s_ksplit, it * NKC + (pidx - NQC) + 1)
                sl = proj_slot(r)
                kb = (jc % 2) * 3 * QUN
                uh = arena_kv[:, kb : kb + QUN]
                ul = arena_kv[:, kb + QUN : kb + 2 * QUN]
                for pos in (0, 64):
                    for dd in range(NDT):
                        terms = ((wkh, uh), (wkh, ul), (wkl, uh))
                        for ti, (W, X) in enumerate(terms):
                            mm = t.matmul(
                                sl[pos : pos + 64, :],
                                W[:, dd * QK : (dd + 1) * QK],
                                X[:, dd * KCH : (dd + 1) * KCH],
                                start=(dd == 0 and ti == 0),
                                stop=(dd == NDT - 1 and ti == 2),
                                tile_position=(0, pos),
                            )
                mm.then_inc(s_kproj, 1)
                # v proj for this chunk's tiles -> ps_v half jc%2
                if jc >= 2:
                    t.wait_ge(s_vtc, it * NKC + jc - 1)
                uv = arena_kv[:, kb + 2 * QUN : kb + 3 * QUN]
                vh = (jc % 2) * 256
                for t4 in range(TPK):
                    for dd in range(NDT):
                        mm = t.matmul(
                            ps_v[:, vh + t4 * 64 : vh + (t4 + 1) * 64],
                            uv[:, dd * KCH + t4 * 128 : dd * KCH + t4 * 128 + 128],
                            wv[:, dd * VD : (dd + 1) * VD],
                            start=(dd == 0),
                            stop=(dd == NDT - 1),
                        )
                mm.then_inc(s_vproj, 1)

            # scores + attn (serial phase: all proj/splits done first)
            t.wait_ge(s_qsplit, it * NQC + NQC)
            t.wait_ge(s_ksplit, it * NKC + NKC)
            for tt in range(NT):
                kt = KS[:, tt * 128 : (tt + 1) * 128]
                for p in range(8):
                    ch = POSC[p]
                    P = pos_slot(p)
                    gp = it * 8 * NT + tt * 8 + p
                    prev = gp - 7 if p == 7 else gp - 2
                    if prev >= 0:
                        t.wait_ge(s_ex, prev + 1)
                    nh = max(1, ICH // 512)
                    w_i = ICH if ICH < 512 else 512
                    for hh in range(nh):
                        cA = ch * ICH + hh * w_i
                        t.matmul(
                            P[:, hh * w_i : (hh + 1) * w_i],
                            kt, Q1[:, cA : cA + w_i],
                            start=True, stop=False,
                        )
                        mm = t.matmul(
                            P[:, hh * w_i : (hh + 1) * w_i],
                            kt, Q2[:, cA : cA + w_i],
                            start=False, stop=True,
                        )
                    mm.then_inc(s_sc, 1)
                    if p == 4 and tt > 0:
                        # attn of tile tt-1
                        t.wait_ge(s_vt2, it * NT + tt)
                        sl_e = slab_e[
                            :, ((tt - 1) % 2) * ICH : ((tt - 1) % 2 + 1) * ICH
                        ]
                        vrow = vt[:, (tt - 1) * VD : tt * VD]
                        for pos in (0, 64):
                            mm2 = t.matmul(
                                ps_o[pos : pos + 64, :],
                                vrow,
                                sl_e[
                                    :, (pos // 64) * (ICH // 2)
                                    : (pos // 64 + 1) * (ICH // 2)
                                ],
                                start=(tt - 1 == 0),
                                stop=False,
                                tile_position=(0, pos),
                                skip_group_check=True,
                            )
                        mm2.then_inc(s_attn, 1)
            # last tile's attn
            t.wait_ge(s_vt2, it * NT + NT)
            sl_e = slab_e[:, ((NT - 1) % 2) * ICH : ((NT - 1) % 2 + 1) * ICH]
            vrow = vt[:, (NT - 1) * VD : NT * VD]
            for pos in (0, 64):
                mm2 = t.matmul(
                    ps_o[pos : pos + 64, :],
                    vrow,
                    sl_e[:, (pos // 64) * (ICH // 2) : (pos // 64 + 1) * (ICH // 2)],
                    start=(NT - 1 == 0),
                    stop=True,
                    tile_position=(0, pos),
                    skip_group_check=True,
                )
            mm2.then_inc(s_attn, 1)

        # ---------------- VECTOR (DVE) ----------------
        cmb_n = [0]

        def combine(v, it, tt):
            c3 = slice(tt * 3, tt * 3 + 3)
            v.wait_ge(s_ex, it * 8 * NT + tt * 8 + 8)
            v.wait_ge(s_e2, it * NT + tt + 1)
            s6 = ssum[:, tt * 8 : tt * 8 + 6].rearrange(
                "p (a two) -> p a two", a=3, two=2
            )

            def step(inst):
                inst.then_inc(s_cmb, 1)
                cmb_n[0] += 1

            def cw():
                v.wait_ge(s_cmb, cmb_n[0])

            # group sums: pairs (0,1) (2,3) (4,5), singles 6 and 7(slab)
            step(v.tensor_tensor(
                spair[:, c3], s6[:, :, 0], s6[:, :, 1], op=ALU.add
            ))
            step(v.tensor_tensor(
                sw2[:, tt * 2 : tt * 2 + 2],
                e_all[:, tt * 5 + 3 : tt * 5 + 5],
                ssum[:, tt * 8 + 6 : tt * 8 + 8], op=ALU.mult,
            ))
            cw()
            step(v.tensor_tensor(
                sw_all[:, c3], e_all[:, tt * 5 : tt * 5 + 3],
                spair[:, c3], op=ALU.mult,
            ))
            cw()
            step(v.tensor_reduce(
                cs_all[:, tt : tt + 1], sw_all[:, c3], axis=AX, op=ALU.add
            ))
            step(v.tensor_reduce(
                cs2[:, tt : tt + 1], sw2[:, tt * 2 : tt * 2 + 2],
                axis=AX, op=ALU.add,
            ))
            cw()
            step(v.tensor_tensor(
                cs_all[:, tt : tt + 1], cs_all[:, tt : tt + 1],
                cs2[:, tt : tt + 1], op=ALU.add,
            ))
            cw()
            step(v.reciprocal(rS_all[:, tt : tt + 1], cs_all[:, tt : tt + 1]))
            cw()
            step(v.tensor_tensor(
                fS_all[:, tt : tt + 1], e_all[:, tt * 5 + 4 : tt * 5 + 5],
                rS_all[:, tt : tt + 1], op=ALU.mult,
            ))
            cw()
            v.wait_ge(s_vtc, it * NKC + tt // TPK + 1)
            v.tensor_scalar_mul(
                vt[:, tt * VD : (tt + 1) * VD],
                vt[:, tt * VD : (tt + 1) * VD],
                fS_all[:, tt : tt + 1],
            ).then_inc(s_vt2, 1)

        @block.vector
        def _(v):
          for it in range(reps):
            # q splits
            for ic in range(NQC):
                v.wait_ge(s_qproj, it * NQC + ic + 1)
                sl = proj_slot(ic)
                cols = slice(ic * QCH, (ic + 1) * QCH)
                v.tensor_copy(Q1[0:64, cols], sl[0:64, :]).then_inc(s_qcp, 1)
                v.tensor_copy(Q2[64:128, cols], sl[64:128, :]).then_inc(s_qcp, 1)
                v.wait_ge(s_qcp, it * 2 * NQC + 2 * ic + 2)
                v.tensor_tensor(
                    Q1[64:128, cols], sl[64:128, :], Q2[64:128, cols],
                    op=ALU.subtract,
                )
                v.tensor_tensor(
                    Q2[0:64, cols], sl[0:64, :], Q1[0:64, cols],
                    op=ALU.subtract,
                ).then_inc(s_qsplit, 1)
            # k splits + v truncs
            for jc in range(NKC):
                v.wait_ge(s_kproj, it * NKC + jc + 1)
                if it * NKC + jc > 0:
                    v.wait_ge(s_ksplit, it * NKC + jc)  # ktmp reuse
                sl = proj_slot(NQC + jc)
                cols = slice(jc * KCH, (jc + 1) * KCH)
                v.tensor_copy(KS[0:64, cols], sl[0:64, :]).then_inc(s_kcp, 1)
                v.tensor_copy(ktmp[64:128, :], sl[64:128, :]).then_inc(s_kcp, 1)
                v.wait_ge(s_kcp, it * 2 * NKC + 2 * jc + 2)
                v.tensor_tensor(
                    KS[64:128, cols], sl[64:128, :], ktmp[64:128, :],
                    op=ALU.subtract,
                ).then_inc(s_ksplit, 1)
                v.wait_ge(s_vproj, it * NKC + jc + 1)
                vh = (jc % 2) * 256
                v.tensor_copy(
                    vt[:, jc * TPK * VD : (jc + 1) * TPK * VD],
                    ps_v[:, vh : vh + TPK * VD],
                ).then_inc(s_vtc, 1)

            for tt in range(NT):
                if tt > 0:
                    combine(v, it, tt - 1)
                for pr in range(3):
                    v.wait_ge(s_sc, it * 8 * NT + tt * 8 + 2 * pr + 2)
                    v.tensor_reduce(
                        nmax[:, tt * 5 + pr : tt * 5 + pr + 1],
                        ps_ab[:, 0 : 2 * ICH], axis=AX, op=ALU.max,
                        negate=True,
                    ).then_inc(s_mx, 1)
                v.wait_ge(s_sc, it * 8 * NT + tt * 8 + 7)
                v.tensor_reduce(
                    nmax[:, tt * 5 + 3 : tt * 5 + 4],
                    ps_ab[:, 0:ICH], axis=AX, op=ALU.max, negate=True,
                ).then_inc(s_mx, 1)
                v.wait_ge(s_sc, it * 8 * NT + tt * 8 + 8)
                v.tensor_reduce(
                    nmax[:, tt * 5 + 4 : tt * 5 + 5],
                    ps_c[:, 0:ICH], axis=AX, op=ALU.max, negate=True,
                ).then_inc(s_mx, 1)
                v.wait_ge(s_mx, it * 5 * NT + tt * 5 + 5)
                v.tensor_reduce(
                    nB[:, tt : tt + 1], nmax[:, tt * 5 : tt * 5 + 5],
                    axis=AX, op=ALU.min,
                ).then_inc(s_d, 1)
            combine(v, it, NT - 1)
            v.wait_ge(s_attn, it * NT + NT)
            v.tensor_copy(out_sb[:, :], ps_o[:, :]).then_inc(s_oc, 1)

        # ---------------- SCALAR (ACT) ----------------
        @block.scalar
        def _(sc):
          for it in range(reps):
            for tt in range(NT):
                for p in range(8):
                    P = pos_slot(p)
                    grp = p // 2 if p < 6 else p - 3
                    sc.wait_ge(s_mx, it * 5 * NT + tt * 5 + grp + 1)
                    bias = nmax[:, tt * 5 + grp : tt * 5 + grp + 1]
                    acc = ssum[:, tt * 8 + p : tt * 8 + p + 1]
                    if p == 7:
                        if it * NT + tt - 1 > 0:
                            sc.wait_ge(s_attn, it * NT + tt - 1)
                        outap = slab_e[:, (tt % 2) * ICH : (tt % 2 + 1) * ICH]
                    else:
                        outap = scr[:, (p % 2) * ICH : (p % 2 + 1) * ICH]
                    sc.activation(
                        outap, P, ACTF.Exp,
                        bias=bias, scale=1.0, accum_out=acc,
                    ).then_inc(s_ex, 1)
                # pair rescale weights e = exp(nB - nmax)
                sc.wait_ge(s_d, it * NT + tt + 1)
                sc.activation(
                    e_all[:, tt * 5 : tt * 5 + 5],
                    nmax[:, tt * 5 : tt * 5 + 5],
                    ACTF.Exp, scale=-1.0, bias=nB[:, tt : tt + 1],
                ).then_inc(s_e2, 1)

    nc.finalize()
    return nc


# ------------------------- host side -------------------------

def _split_bf16(x):
    import ml_dtypes

    hi = x.astype(ml_dtypes.bfloat16)
    lo = (x - hi.astype(np.float32)).astype(ml_dtypes.bfloat16)
    return hi, lo


def _tile_cols(xT, w):
    dd = xT.shape[0] // 128
    return np.ascontiguousarray(
        xT.reshape(dd, 128, w).transpose(1, 0, 2).reshape(128, dd * w)
    )


def _tile_chunked(xT, ch):
    """[d, s] -> [128, (s/ch)*(d/128)*ch], col = jc*(nd*ch) + dd*ch + jj."""
    d, s = xT.shape
    nd = d // 128
    njc = s // ch
    a = xT.reshape(nd, 128, njc, ch).transpose(1, 2, 0, 3)
    return np.ascontiguousarray(a.reshape(128, njc * nd * ch))


def build_in_maps2(inputs, seq=8192, d=1024):
    import ml_dtypes

    bf = ml_dtypes.bfloat16
    SLAB = seq // C
    QCH = 512
    KCH = 512
    qw8 = (inputs["query_weights"] / np.sqrt(np.float32(QK))).astype(np.float32)
    wqh, wql = _split_bf16(qw8)
    wkh, wkl = _split_bf16(inputs["key_weights"].astype(np.float32))
    wv = inputs["value_weights"].astype(bf)
    w_tiled = {
        "wqh": _tile_cols(wqh.astype(np.float32), QK).astype(bf),
        "wql": _tile_cols(wql.astype(np.float32), QK).astype(bf),
        "wkh": _tile_cols(wkh.astype(np.float32), QK).astype(bf),
        "wkl": _tile_cols(wkl.astype(np.float32), QK).astype(bf),
        "wv": _tile_cols(wv.astype(np.float32), VD).astype(bf),
    }
    nd = d // 128
    NQC = seq // QCH
    NKC = seq // KCH
    QUN = nd * QCH

    def interleave(parts, nch):
        """each part [128, nch*QUN] chunk-major -> [128, nch*len(parts)*QUN]
        with chunk ic holding [part0 | part1 | ...]."""
        stacked = np.concatenate(
            [p.reshape(128, nch, QUN) for p in parts], axis=2
        )
        return np.ascontiguousarray(
            stacked.reshape(128, nch * len(parts) * QUN)
        )

    kT = np.ascontiguousarray(inputs["keys"].T).astype(np.float32)
    vT = np.ascontiguousarray(inputs["values"].T).astype(np.float32)
    kh, kl = _split_bf16(kT)
    kv_int = interleave(
        [
            _tile_chunked(kh.astype(np.float32), KCH).astype(bf),
            _tile_chunked(kl.astype(np.float32), KCH).astype(bf),
            _tile_chunked(vT.astype(np.float32), KCH).astype(bf),
        ],
        NKC,
    )
    qT = np.ascontiguousarray(inputs["queries"].T).astype(np.float32)
    in_maps = []
    for c in range(C):
        qrot = np.roll(qT, -c * SLAB, axis=1)
        qh, ql = _split_bf16(qrot)
        q_int = interleave(
            [
                _tile_chunked(qh.astype(np.float32), QCH).astype(bf),
                _tile_chunked(ql.astype(np.float32), QCH).astype(bf),
            ],
            NQC,
        )
        m = {"qin": q_int, "kvin": kv_int}
        m.update(w_tiled)
        in_maps.append(m)
    return in_maps


def assemble_out2(results, seq=8192):
    SLAB = seq // C
    full = np.zeros((seq, VD), np.float32)
    for c in range(C):
        o = np.asarray(results[c]["out"], dtype=np.float32)
        slab = np.concatenate([o[0:64, :], o[64:128, :]], axis=1).T
        full[c * SLAB : (c + 1) * SLAB] = slab
    return full




def run_spmd_staged(nc, in_maps, profile_dir=None):
    """run_bass_via_pjrt with inputs pre-staged on-device (blocks until all
    shards are resident) so the 8 cores launch aligned instead of staggered
    by per-device input-transfer time. Optionally wraps the execute in the
    axon NTFF profile hook (profile_dir)."""
    import jax
    import numpy as np_
    from jax.sharding import Mesh, PartitionSpec, NamedSharding
    from jax.experimental.shard_map import shard_map
    import concourse.mybir as mybir
    from concourse import bass2jax

    bass2jax.install_neuronx_cc_hook()
    n_cores = len(in_maps)

    partition_name = (
        nc.partition_id_tensor.name if nc.partition_id_tensor else None
    )
    in_names, out_names, out_avals, zero_outs = [], [], [], []
    for alloc in nc.m.functions[0].allocations:
        if not isinstance(alloc, mybir.MemoryLocationSet):
            continue
        name = alloc.memorylocations[0].name
        if alloc.kind == "ExternalInput":
            if name != partition_name:
                in_names.append(name)
        elif alloc.kind == "ExternalOutput":
            out_names.append(name)
            shape = tuple(alloc.tensor_shape)
            dtype = mybir.dt.np(alloc.dtype)
            out_avals.append(jax.core.ShapedArray(shape, dtype))
            zero_outs.append(np_.zeros(shape, dtype))
    n_params = len(in_names)
    n_outs = len(out_avals)
    all_names = in_names + out_names
    if partition_name is not None:
        all_names = all_names + [partition_name]

    def _body(*args):
        operands = list(args)
        if partition_name is not None:
            operands.append(bass2jax.partition_id_tensor())
        outs = bass2jax._bass_exec_p.bind(
            *operands,
            out_avals=tuple(out_avals),
            in_names=tuple(all_names),
            out_names=tuple(out_names),
            lowering_input_output_aliases=(),
            sim_require_finite=True,
            sim_require_nnan=True,
            nc=nc,
        )
        return tuple(outs)

    devices = jax.devices()[:n_cores]
    mesh = Mesh(np_.asarray(devices), ("core",))
    spec = NamedSharding(mesh, PartitionSpec("core"))
    sharded = jax.jit(
        shard_map(
            _body,
            mesh=mesh,
            in_specs=(PartitionSpec("core"),) * (n_params + n_outs),
            out_specs=(PartitionSpec("core"),) * n_outs,
            check_rep=False,
        ),
        donate_argnums=tuple(range(n_params, n_params + n_outs)),
        keep_unused=True,
    )
    concat_in = [
        np_.concatenate([np_.asarray(in_maps[c][nm]) for c in range(n_cores)], axis=0)
        for nm in in_names
    ]
    concat_zero = [
        np_.zeros((n_cores * z.shape[0], *z.shape[1:]), z.dtype) for z in zero_outs
    ]
    staged = [jax.device_put(a, spec) for a in concat_in + concat_zero]
    jax.block_until_ready(staged)

    if profile_dir is not None:
        from antenv.axon_hooks import get_axon_ntff_profile_hook

        hook = get_axon_ntff_profile_hook()
        with hook(profile_dir, list(range(n_cores))):
            out_arrs = sharded(*staged)
            jax.block_until_ready(out_arrs)
    else:
        out_arrs = sharded(*staged)
    return [
        {
            nm: np_.asarray(out_arrs[i]).reshape(n_cores, *out_avals[i].shape)[c]
            for i, nm in enumerate(out_names)
        }
        for c in range(n_cores)
    ]




def kernel(queries, keys, values, query_weights, key_weights, value_weights):
    import sys

    for p in ("/opt/trn_rl_repo",):
        if p not in sys.path:
            sys.path.insert(0, p)

    seq, d = queries.shape
    inputs = {
        "queries": queries, "keys": keys, "values": values,
        "query_weights": query_weights, "key_weights": key_weights,
        "value_weights": value_weights,
    }
    in_maps = build_in_maps2(inputs, seq=seq, d=d)
    nc = build_nc2(seq=seq, d=d)
    results = run_spmd_staged(nc, in_maps)
    return assemble_out2(results, seq=seq)



# revision 17
# speedup vs baseline: 6.3684x; 6.3684x over previous
"""Sequence-parallel Trainium2 attention-head kernel (softmax over queries).

Shard the QUERY dim across the 8 cores (slab = 1024 queries each); every
core computes scores for ALL 8192 keys x its own query slab with j (keys)
on partitions, so the softmax-over-queries stats are per-partition-row:

  out[i,:] = sum_j exp(s_ij - M_j)/D_j * v[j,:],  M_j/D_j global over i.

Per core:
  1. Project its k/v slab (j on partitions) -> AllGather #1 (384 KB/core)
     so every core holds full projected KS=[kh;kl] and v.  Project its q
     slab -> Q1=[qh;ql], Q2=[ql;qh] (weights host-duplicated into both
     column halves so the fp32 projection lands on all 128 partitions in
     one pass; hi/lo split is then partition-aligned DVE).
  2. Per j-tile t (128 keys x 1024 local queries): 2 stacked matmuls
     (KS_t^T Q1 + KS_t^T Q2 = all 4 hi/lo cross terms), DVE negated
     row-max -> bias, ACT exp (psum->bf16) with accum_out -> local sums.
     Bias is the LOCAL max over this core's slab, so the per-row rescale
     exp(b_local - M)/D folds entirely into v later.
  3. AllGather #2 of the per-(j,core) stats [negmax|sum] (64 KB/core);
     every core reduces them to global M_j, D_j, rescales v rows by
     f_j = exp(b_local_j - M_j)/D_j, then accumulates the 64 attn
     matmuls v'_t^T e_t into psum [64 v, 1024 i] and writes its slab.
"""

import numpy as np

C = 8
QK = 64
VD = 64


def build_nc2(seq=8192, d=1024, reps=1, warmup=True, dbg=False, noscale=False):
    import concourse.bacc as bacc
    import concourse.mybir as mybir

    f32 = mybir.dt.float32
    bf16 = mybir.dt.bfloat16
    AX = mybir.AxisListType.X
    ALU = mybir.AluOpType
    ACTF = mybir.ActivationFunctionType

    NDT = d // 128            # 8 d-tiles
    SLAB = seq // C           # 1024 queries per core
    NT = seq // 128           # 64 global j-tiles
    NST = SLAB // 128         # 8 v-proj subtiles per slab
    SS = max(SLAB, 512)       # psum slot stride (bank-aligned)
    NDMA = 3 * NDT            # arena chunk DMAs per iteration (k, v, q)

    nc = bacc.Bacc(target_bir_lowering=False, debug=False, num_devices=C)

    def din(name, w, dt=bf16):
        return nc.declare_dram_parameter(name, [128, w], dt, isOutput=False)

    qin_d = din("qin", NDT * 2 * SLAB)     # col = dd*2048 + h*1024 + i
    kin_d = din("kin", NDT * 2 * SLAB)
    vin_d = din("vin", NDT * SLAB)         # col = dd*1024 + j
    wq2h_d, wq2l_d = din("wq2h", NDT * 128), din("wq2l", NDT * 128)
    wk2h_d, wk2l_d = din("wk2h", NDT * 128), din("wk2l", NDT * 128)
    wv_d = din("wv", NDT * VD)
    out_d = nc.declare_dram_parameter("out", [VD, SLAB], f32, isOutput=True)
    if dbg:
        dKS_d = nc.declare_dram_parameter("dKS", [128, seq], bf16, isOutput=True)
        dQ1_d = nc.declare_dram_parameter("dQ1", [128, SLAB], bf16, isOutput=True)
        dQ2_d = nc.declare_dram_parameter("dQ2", [128, SLAB], bf16, isOutput=True)
        dst_d = nc.declare_dram_parameter("dst", [128, 2 * NT], f32, isOutput=True)
        dsa_d = nc.declare_dram_parameter("dsa", [128, C * 2 * NT], f32, isOutput=True)
        df_d = nc.declare_dram_parameter("df", [128, NT], f32, isOutput=True)
        dvl_d = nc.declare_dram_parameter("dvl", [128, SLAB // 2], bf16, isOutput=True)
        dva_d = nc.declare_dram_parameter("dva", [128, NT * VD], bf16, isOutput=True)
        de_d = nc.declare_dram_parameter("de", [128, 4 * SLAB], bf16, isOutput=True)

    # collective bounce buffers (DRAM; collectives can't touch I/O tensors)
    cc1_in = nc.dram_tensor("cc1_in", [128, SLAB + SLAB // 2], bf16)
    cc1_out = nc.dram_tensor(
        "cc1_out", [C * 128, SLAB + SLAB // 2], bf16, addr_space="Shared"
    )
    cc2_in = nc.dram_tensor("cc2_in", [128, 2 * NT], f32)
    cc2_out = nc.dram_tensor("cc2_out", [C * 128, 2 * NT], f32, addr_space="Shared")

    from contextlib import ExitStack

    with ExitStack() as ctx:
        block = ctx.enter_context(nc.Block())
        sem = lambda n: ctx.enter_context(nc.semaphore(n))
        sb = lambda n, shape, dt: ctx.enter_context(nc.sbuf_tensor(n, shape, dt))
        ps = lambda n, shape: ctx.enter_context(nc.psum_tensor(n, shape, f32))

        s_w = sem("s_w")        # weight DMAs: 80 once
        s_in = sem("s_in")      # k/v arena DMAs: 16*16/iter (serialized)
        s_inq = sem("s_inq")    # q arena DMAs: 8*16/iter (serialized)
        s_kp = sem("s_kp")      # 8/iter (k proj per dd)
        s_vp = sem("s_vp")      # 8/iter
        s_qp = sem("s_qp")      # 8/iter
        s_ks = sem("s_ks")      # 1/iter k split done
        s_vsp = sem("s_vsp")    # 1/iter vloc copy done
        s_qs = sem("s_qs")      # 1/iter q split done
        s_gb = sem("s_gb")      # 32/iter bounce-in DMAs
        s_cc1 = sem("s_cc1")    # 1/iter
        s_ccd = sem("s_ccd")    # 256/iter gather-back DMAs
        s_sc = sem("s_sc")      # 64/iter score tiles
        s_mx = sem("s_mx")      # 64/iter
        s_ex = sem("s_ex")      # 64/iter
        s_g2 = sem("s_g2")      # 16/iter stats bounce DMA
        s_cc2 = sem("s_cc2")    # 1/iter
        s_std = sem("s_std")    # 128/iter stats gather-back
        s_sm = sem("s_sm")      # 1/iter DVE->ACT stats handoff
        s_sa = sem("s_sa")      # 1/iter ACT->DVE stats handoff
        s_vsc = sem("s_vsc")    # 64/iter v rescales
        s_at = sem("s_at")      # 1/iter attn done
        s_oc = sem("s_oc")      # 1/iter out copy done
        s_out = sem("s_out")    # 16/iter out DMA
        s_ch = sem("s_ch")      # DVE same-engine RAW chain

        arena_k = sb("arena_k", [128, 2 * 2 * SLAB], bf16)  # dd%2 ping-pong
        arena_v = sb("arena_v", [128, 2 * NDT * 128], bf16)
        arena_q = sb("arena_q", [128, 2 * 2 * SLAB], bf16)
        wq2h = sb("wq2h_s", [128, NDT * 128], bf16)
        wq2l = sb("wq2l_s", [128, NDT * 128], bf16)
        wk2h = sb("wk2h_s", [128, NDT * 128], bf16)
        wk2l = sb("wk2l_s", [128, NDT * 128], bf16)
        wv = sb("wv_s", [128, NDT * VD], bf16)
        Q1 = sb("Q1", [128, SLAB], bf16)
        Q2 = sb("Q2", [128, SLAB], bf16)
        KSloc = sb("KSloc", [128, SLAB], bf16)
        ktmp = sb("ktmp", [128, SLAB], bf16)
        vloc = sb("vloc", [128, SLAB // 2], bf16)
        KS = sb("KS", [128, seq], bf16)
        v_all = sb("v_all", [128, NT * VD], bf16)
        e_sb = sb("e_sb", [128, NT * SLAB], bf16)           # 128 KB/part
        stats_loc = sb("stats_loc", [128, 2 * NT], f32)     # [negmax | sum]
        stats_all = sb("stats_all", [128, C * 2 * NT], f32)
        NM = sb("NM", [128, NT], f32)
        darg = sb("darg", [128, C * NT], f32)
        w8 = sb("w8", [128, C * NT], f32)
        wD = sb("wD", [128, C * NT], f32)
        Dt = sb("Dt", [128, NT], f32)
        Rt = sb("Rt", [128, NT], f32)
        wlarg = sb("wlarg", [128, NT], f32)
        wl = sb("wl", [128, NT], f32)
        f_sb = sb("f_sb", [128, NT], f32)
        out_sb = sb("out_sb", [VD, SLAB], f32)

        ps_s = ps("ps_s", [128, 2 * SS])     # proj + score slots (4 banks)
        ps_v = ps("ps_v", [128, SLAB // 2])  # 1 bank
        ps_o = ps("ps_o", [VD, SLAB])        # 2 banks

        # ---------------- SYNC: k/v input + output DMAs ----------------
        # per-chunk s_in levels must be ordered sync points, so each arena
        # DMA is serialized (waited) before the next is issued.
        @block.sync
        def _(s):
          for it in range(reps):
            n_in = it * (NDT + NST)
            if it == 0:
                for wsb, wdr in ((wq2h, wq2h_d), (wq2l, wq2l_d),
                                 (wk2h, wk2h_d), (wk2l, wk2l_d), (wv, wv_d)):
                    s.dma_start(out=wsb[:, :], in_=wdr[:, :]).then_inc(s_w, 16)
            for dd in range(NDT):          # k chunks
                g = it * NDT + dd
                if g >= 2:
                    s.wait_ge(s_kp, g - 1)
                s.dma_start(
                    out=arena_k[:, (dd % 2) * 2 * SLAB : (dd % 2 + 1) * 2 * SLAB],
                    in_=kin_d[:, dd * 2 * SLAB : (dd + 1) * 2 * SLAB],
                ).then_inc(s_in, 16)
                n_in += 1
                s.wait_ge(s_in, n_in * 16)
            for st in range(NST):          # v chunks (one j-subtile, all dd)
                g = it * NST + st
                if g >= 2:
                    s.wait_ge(s_vp, g - 1)
                s.dma_start(
                    out=arena_v[:, (st % 2) * NDT * 128 : (st % 2 + 1) * NDT * 128],
                    in_=vin_d[:, st * NDT * 128 : (st + 1) * NDT * 128],
                ).then_inc(s_in, 16)
                n_in += 1
                s.wait_ge(s_in, n_in * 16)
            s.wait_ge(s_oc, it + 1)
            s.dma_start(out=out_d[:, :], in_=out_sb[:, :]).then_inc(s_out, 16)
            s.wait_ge(s_out, it * 16 + 16)
            if dbg and it == reps - 1:
                n_d = 0
                for dst, srcb in ((dKS_d, KS), (dQ1_d, Q1), (dQ2_d, Q2),
                                  (dst_d, stats_loc), (dsa_d, stats_all),
                                  (df_d, f_sb), (dva_d, v_all),
                                  (dvl_d, vloc)):
                    s.dma_start(out=dst[:, :], in_=srcb[:, :]).then_inc(s_out, 16)
                    n_d += 1
                s.dma_start(
                    out=de_d[:, :], in_=e_sb[:, 0 : 4 * SLAB]
                ).then_inc(s_out, 16)
                n_d += 1
                s.wait_ge(s_out, it * 16 + 16 + n_d * 16)

        # ---------------- TENSOR (PE) ----------------
        @block.tensor
        def _(t):
          for it in range(reps):
            for w_ in range(40 if (warmup and it == 0) else 0):
                t.matmul(
                    ps_s[0:64, 0:512], Q1[:, 0:64], Q1[:, 0 : min(SLAB, 512)],
                    start=(w_ == 0), stop=False,
                )
            if it == 0:
                t.wait_ge(s_w, 80)
            # ---- k proj -> ps_s[:, 0:SLAB] (both halves via dup weights)
            if it > 0:
                t.wait_ge(s_ex, it * NT - 1)   # slot0 free (prev tile 62)
            for dd in range(NDT):
                t.wait_ge(s_in, (it * (NDT + NST) + dd + 1) * 16)
                kb = (dd % 2) * 2 * SLAB
                uh = arena_k[:, kb : kb + SLAB]
                ul = arena_k[:, kb + SLAB : kb + 2 * SLAB]
                for ti, (W, X) in enumerate(((wk2h, uh), (wk2h, ul), (wk2l, uh))):
                    for hb in range(0, SLAB, 512):
                        he = min(hb + 512, SLAB)
                        mm = t.matmul(
                            ps_s[:, hb:he],
                            W[:, dd * 128 : (dd + 1) * 128], X[:, hb:he],
                            start=(dd == 0 and ti == 0),
                            stop=(dd == NDT - 1 and ti == 2),
                            skip_group_check=True,
                        )
                mm.then_inc(s_kp, 1)
            # ---- v proj -> ps_v (st-outer: one sequential group per
            # subtile; vin is re-tiled so chunk st holds all dd for its j's)
            if it > 0:
                t.wait_ge(s_vsp, it)           # ps_v free
            for st in range(NST):
                t.wait_ge(s_in, (it * (NDT + NST) + NDT + st + 1) * 16)
                vb = (st % 2) * NDT * 128
                for dd in range(NDT):
                    mm = t.matmul(
                        ps_v[:, st * VD : (st + 1) * VD],
                        arena_v[:, vb + dd * 128 : vb + (dd + 1) * 128],
                        wv[:, dd * VD : (dd + 1) * VD],
                        start=(dd == 0), stop=(dd == NDT - 1),
                    )
                mm.then_inc(s_vp, 1)
            # ---- q proj -> ps_s[:, SLAB:2*SLAB]
            if it > 0:
                t.wait_ge(s_ex, it * NT)       # slot1 free (prev tile 63)
            for dd in range(NDT):
                t.wait_ge(s_inq, (it * NDT + dd + 1) * 16)
                qb = (dd % 2) * 2 * SLAB
                uh = arena_q[:, qb : qb + SLAB]
                ul = arena_q[:, qb + SLAB : qb + 2 * SLAB]
                for ti, (W, X) in enumerate(((wq2h, uh), (wq2h, ul), (wq2l, uh))):
                    for hb in range(0, SLAB, 512):
                        he = min(hb + 512, SLAB)
                        mm = t.matmul(
                            ps_s[:, SS + hb : SS + he],
                            W[:, dd * 128 : (dd + 1) * 128], X[:, hb:he],
                            start=(dd == 0 and ti == 0),
                            stop=(dd == NDT - 1 and ti == 2),
                            skip_group_check=True,
                        )
                mm.then_inc(s_qp, 1)
            # ---- scores: 64 j-tiles
            t.wait_ge(s_ccd, (it + 1) * 256)
            t.wait_ge(s_qs, it + 1)
            for tt in range(NT):
                g = it * NT + tt
                if g >= 2:
                    t.wait_ge(s_ex, g - 1)     # slot free (exp of tt-2)
                so = (tt % 2) * SS
                kt = KS[:, tt * 128 : (tt + 1) * 128]
                for hb in range(0, SLAB, 512):
                    he = min(hb + 512, SLAB)
                    t.matmul(
                        ps_s[:, so + hb : so + he], kt, Q1[:, hb:he],
                        start=True, stop=False, skip_group_check=True,
                    )
                    mm = t.matmul(
                        ps_s[:, so + hb : so + he], kt, Q2[:, hb:he],
                        start=False, stop=True, skip_group_check=True,
                    )
                mm.then_inc(s_sc, 1)
            # ---- attn: accumulate 64 tiles into ps_o
            t.wait_ge(s_oc, it)                # ps_o free
            for tt in range(NT):
                t.wait_ge(s_vsc, it * NT + tt + 1)
                for hb in range(0, SLAB, 512):
                    he = min(hb + 512, SLAB)
                    mm = t.matmul(
                        ps_o[:, hb:he],
                        v_all[:, tt * VD : (tt + 1) * VD],
                        e_sb[:, tt * SLAB + hb : tt * SLAB + he],
                        start=(tt == 0), stop=(tt == NT - 1),
                        skip_group_check=True,
                    )
            mm.then_inc(s_at, 1)

        # ---------------- VECTOR (DVE) ----------------
        ch_n = [0]

        @block.vector
        def _(v):
          def step(inst):
              inst.then_inc(s_ch, 1)
              ch_n[0] += 1

          def cw(v):
              v.wait_ge(s_ch, ch_n[0])

          for it in range(reps):
            # k split -> KSloc = [kh; kl]
            v.wait_ge(s_kp, it * NDT + NDT)
            if it > 0:
                v.wait_ge(s_gb, it * 32)       # KSloc/vloc consumed
            step(v.tensor_copy(KSloc[0:64, :], ps_s[0:64, 0:SLAB]))
            step(v.tensor_copy(ktmp[64:128, :], ps_s[64:128, 0:SLAB]))
            cw(v)
            v.tensor_tensor(
                KSloc[64:128, :], ps_s[64:128, 0:SLAB], ktmp[64:128, :],
                op=ALU.subtract,
            ).then_inc(s_ks, 1)
            # v copy
            v.wait_ge(s_vp, it * NST + NST)
            v.tensor_copy(vloc[:, :], ps_v[:, :]).then_inc(s_vsp, 1)
            # q split -> Q1=[qh;ql], Q2=[ql;qh]
            v.wait_ge(s_qp, it * NDT + NDT)
            qsl = ps_s[:, SS : SS + SLAB]
            step(v.tensor_copy(Q1[0:64, :], qsl[0:64, :]))
            step(v.tensor_copy(Q2[64:128, :], qsl[64:128, :]))
            cw(v)
            v.tensor_tensor(
                Q1[64:128, :], qsl[64:128, :], Q2[64:128, :], op=ALU.subtract
            )
            v.tensor_tensor(
                Q2[0:64, :], qsl[0:64, :], Q1[0:64, :], op=ALU.subtract
            ).then_inc(s_qs, 1)
            # per-tile negated row-max
            for tt in range(NT):
                v.wait_ge(s_sc, it * NT + tt + 1)
                if tt == 0 and it > 0:
                    v.wait_ge(s_g2, it * 16)   # stats_loc consumed
                v.tensor_reduce(
                    stats_loc[:, tt : tt + 1],
                    ps_s[:, (tt % 2) * SS : (tt % 2) * SS + SLAB],
                    axis=AX, op=ALU.max, negate=True,
                ).then_inc(s_mx, 1)
            # stats math
            v.wait_ge(s_std, (it + 1) * 128)
            nb_v = stats_all[:, :].rearrange("p (c t) -> p t c", c=C, t=2 * NT)
            step(v.tensor_reduce(NM[:, :], nb_v[:, 0:NT, :], axis=AX, op=ALU.min))
            cw(v)
            for c in range(C):
                v.tensor_tensor(
                    darg[:, c * NT : (c + 1) * NT], NM[:, :],
                    stats_all[:, c * 2 * NT : c * 2 * NT + NT],
                    op=ALU.subtract,
                )
            v.tensor_tensor(
                wlarg[:, :], NM[:, :], stats_loc[:, 0:NT], op=ALU.subtract
            ).then_inc(s_sm, 1)
            v.wait_ge(s_sa, it + 1)
            dl_v = stats_all[:, :].rearrange("p (c t) -> p c t", c=C, t=2 * NT)
            step(v.tensor_tensor(
                wD[:, :].rearrange("p (c t) -> p c t", c=C, t=NT),
                w8[:, :].rearrange("p (c t) -> p c t", c=C, t=NT),
                dl_v[:, :, NT : 2 * NT],
                op=ALU.mult,
            ))
            cw(v)
            step(v.tensor_reduce(
                Dt[:, :],
                wD[:, :].rearrange("p (c t) -> p t c", c=C, t=NT),
                axis=AX, op=ALU.add,
            ))
            cw(v)
            step(v.reciprocal(Rt[:, :], Dt[:, :]))
            cw(v)
            step(v.tensor_tensor(f_sb[:, :], wl[:, :], Rt[:, :], op=ALU.mult))
            cw(v)
            # rescale v rows (in place)
            v.wait_ge(s_ccd, (it + 1) * 256)
            for tt in range(NT):
                v.tensor_scalar_mul(
                    v_all[:, tt * VD : (tt + 1) * VD],
                    v_all[:, tt * VD : (tt + 1) * VD],
                    1.0 if noscale else f_sb[:, tt : tt + 1],
                ).then_inc(s_vsc, 1)

        # ---------------- SCALAR (ACT) ----------------
        @block.scalar
        def _(sc):
          for it in range(reps):
            # q arena stream on the ACT queue (serialized per-chunk levels)
            for dd in range(NDT):
                g = it * NDT + dd
                if g >= 2:
                    sc.wait_ge(s_qp, g - 1)
                sc.dma_start(
                    out=arena_q[:, (dd % 2) * 2 * SLAB : (dd % 2 + 1) * 2 * SLAB],
                    in_=qin_d[:, dd * 2 * SLAB : (dd + 1) * 2 * SLAB],
                ).then_inc(s_inq, 16)
                sc.wait_ge(s_inq, (it * NDT + dd + 1) * 16)
            for tt in range(NT):
                sc.wait_ge(s_mx, it * NT + tt + 1)
                if tt == 0 and it > 0:
                    sc.wait_ge(s_at, it)       # e_sb consumed by attn
                sc.activation(
                    e_sb[:, tt * SLAB : (tt + 1) * SLAB],
                    ps_s[:, (tt % 2) * SS : (tt % 2) * SS + SLAB],
                    ACTF.Exp,
                    bias=stats_loc[:, tt : tt + 1], scale=1.0,
                    accum_out=stats_loc[:, NT + tt : NT + tt + 1],
                ).then_inc(s_ex, 1)
            sc.wait_ge(s_sm, it + 1)
            sc.activation(w8[:, :], darg[:, :], ACTF.Exp)
            sc.activation(wl[:, :], wlarg[:, :], ACTF.Exp).then_inc(s_sa, 1)
            sc.wait_ge(s_at, it + 1)
            if it > 0:
                sc.wait_ge(s_out, it * 16)     # out_sb consumed
            sc.activation(out_sb[:, :], ps_o[:, :], ACTF.Copy).then_inc(s_oc, 1)

        # ---------------- GPSIMD: collectives ----------------
        @block.gpsimd
        def _(g):
          for it in range(reps):
            g.wait_ge(s_ks, it + 1)
            g.wait_ge(s_vsp, it + 1)
            if it > 0:
                g.wait_ge(s_cc1, it)           # cc1_in consumed
            g.dma_start(out=cc1_in[:, 0:SLAB], in_=KSloc[:, :]).then_inc(s_gb, 16)
            g.dma_start(
                out=cc1_in[:, SLAB : SLAB + SLAB // 2], in_=vloc[:, :]
            ).then_inc(s_gb, 16)
            g.wait_ge(s_gb, it * 32 + 32)
            g.collective_compute(
                "AllGather", mybir.AluOpType.bypass,
                replica_groups=[list(range(C))],
                ins=[cc1_in[:, :].opt()],
                outs=[cc1_out[:, :].opt()],
            ).then_inc(s_cc1, 1)
            g.wait_ge(s_cc1, it + 1)
            for c in range(C):
                g.dma_start(
                    out=KS[:, c * SLAB : (c + 1) * SLAB],
                    in_=cc1_out[c * 128 : (c + 1) * 128, 0:SLAB],
                ).then_inc(s_ccd, 16)
            for c in range(C):
                g.dma_start(
                    out=v_all[:, c * (SLAB // 2) : (c + 1) * (SLAB // 2)],
                    in_=cc1_out[c * 128 : (c + 1) * 128, SLAB : SLAB + SLAB // 2],
                ).then_inc(s_ccd, 16)
            # stats gather
            g.wait_ge(s_ex, (it + 1) * NT)
            if it > 0:
                g.wait_ge(s_cc2, it)
            g.dma_start(out=cc2_in[:, :], in_=stats_loc[:, :]).then_inc(s_g2, 16)
            g.wait_ge(s_g2, (it + 1) * 16)
            g.collective_compute(
                "AllGather", mybir.AluOpType.bypass,
                replica_groups=[list(range(C))],
                ins=[cc2_in[:, :].opt()],
                outs=[cc2_out[:, :].opt()],
            ).then_inc(s_cc2, 1)
            g.wait_ge(s_cc2, it + 1)
            for c in range(C):
                g.dma_start(
                    out=stats_all[:, c * 2 * NT : (c + 1) * 2 * NT],
                    in_=cc2_out[c * 128 : (c + 1) * 128, :],
                ).then_inc(s_std, 16)

    nc.finalize()
    return nc


# ------------------------- host side -------------------------

def _split_bf16(x):
    import ml_dtypes

    hi = x.astype(ml_dtypes.bfloat16)
    lo = (x - hi.astype(np.float32)).astype(ml_dtypes.bfloat16)
    return hi, lo


def _tile_cols(xT, w):
    """[d, s] -> [128, (d/128)*w] with col = dd*w + i (s == w per d-tile)."""
    dd = xT.shape[0] // 128
    return np.ascontiguousarray(
        xT.reshape(dd, 128, w).transpose(1, 0, 2).reshape(128, dd * w)
    )


def build_in_maps2(inputs, seq=8192, d=1024):
    import ml_dtypes

    bf = ml_dtypes.bfloat16
    SLAB = seq // C
    NDT = d // 128

    qw = (inputs["query_weights"] / np.sqrt(np.float32(QK))).astype(np.float32)
    wqh, wql = _split_bf16(qw)
    wkh, wkl = _split_bf16(inputs["key_weights"].astype(np.float32))

    def dup_tile(w):
        w2 = np.concatenate([w.astype(np.float32)] * 2, axis=1)  # [d, 128]
        return _tile_cols(w2, 128).astype(bf)

    w_maps = {
        "wq2h": dup_tile(wqh), "wq2l": dup_tile(wql),
        "wk2h": dup_tile(wkh), "wk2l": dup_tile(wkl),
        "wv": _tile_cols(
            inputs["value_weights"].astype(np.float32), VD
        ).astype(bf),
    }

    def slab_hi_lo(xT):
        """[d, SLAB] -> [128, NDT*2*SLAB] with col = dd*2*SLAB + h*SLAB + i."""
        hi, lo = _split_bf16(xT)
        ht = _tile_cols(hi.astype(np.float32), SLAB).reshape(128, NDT, SLAB)
        lt = _tile_cols(lo.astype(np.float32), SLAB).reshape(128, NDT, SLAB)
        return np.ascontiguousarray(
            np.concatenate([ht[:, :, None, :], lt[:, :, None, :]], axis=2)
            .reshape(128, NDT * 2 * SLAB)
        ).astype(bf)

    def _vin_tile(xT):
        """[d, SLAB] -> [128, NST*NDT*128], col = st*NDT*128 + dd*128 + jj."""
        t = _tile_cols(xT, SLAB).reshape(128, NDT, SLAB // 128, 128)
        return np.ascontiguousarray(
            t.transpose(0, 2, 1, 3).reshape(128, -1)
        )

    qT = np.ascontiguousarray(inputs["queries"].T).astype(np.float32)
    kT = np.ascontiguousarray(inputs["keys"].T).astype(np.float32)
    vT = np.ascontiguousarray(inputs["values"].T).astype(np.float32)

    in_maps = []
    for c in range(C):
        sl = slice(c * SLAB, (c + 1) * SLAB)
        m = {
            "qin": slab_hi_lo(qT[:, sl]),
            "kin": slab_hi_lo(kT[:, sl]),
            "vin": _vin_tile(vT[:, sl]).astype(bf),
        }
        m.update(w_maps)
        in_maps.append(m)
    return in_maps


def assemble_out2(results, seq=8192):
    SLAB = seq // C
    full = np.zeros((seq, VD), np.float32)
    for c in range(C):
        o = np.asarray(results[c]["out"], dtype=np.float32)  # [VD, SLAB]
        full[c * SLAB : (c + 1) * SLAB] = o.T
    return full


def run_spmd_staged(nc, in_maps, profile_dir=None):
    """run_bass_via_pjrt with inputs pre-staged on-device (blocks until all
    shards are resident) so the 8 cores launch aligned instead of staggered
    by per-device input-transfer time."""
    import jax
    import numpy as np_
    from jax.sharding import Mesh, PartitionSpec, NamedSharding
    from jax.experimental.shard_map import shard_map
    import concourse.mybir as mybir
    from concourse import bass2jax

    bass2jax.install_neuronx_cc_hook()
    n_cores = len(in_maps)

    partition_name = (
        nc.partition_id_tensor.name if nc.partition_id_tensor else None
    )
    in_names, out_names, out_avals, zero_outs = [], [], [], []
    for alloc in nc.m.functions[0].allocations:
        if not isinstance(alloc, mybir.MemoryLocationSet):
            continue
        name = alloc.memorylocations[0].name
        if alloc.kind == "ExternalInput":
            if name != partition_name:
                in_names.append(name)
        elif alloc.kind == "ExternalOutput":
            out_names.append(name)
            shape = tuple(alloc.tensor_shape)
            dtype = mybir.dt.np(alloc.dtype)
            out_avals.append(jax.core.ShapedArray(shape, dtype))
            zero_outs.append(np_.zeros(shape, dtype))
    n_params = len(in_names)
    n_outs = len(out_avals)
    all_names = in_names + out_names
    if partition_name is not None:
        all_names = all_names + [partition_name]

    def _body(*args):
        operands = list(args)
        if partition_name is not None:
            operands.append(bass2jax.partition_id_tensor())
        outs = bass2jax._bass_exec_p.bind(
            *operands,
            out_avals=tuple(out_avals),
            in_names=tuple(all_names),
            out_names=tuple(out_names),
            lowering_input_output_aliases=(),
            sim_require_finite=True,
            sim_require_nnan=True,
            nc=nc,
        )
        return tuple(outs)

    devices = jax.devices()[:n_cores]
    mesh = Mesh(np_.asarray(devices), ("core",))
    spec = NamedSharding(mesh, PartitionSpec("core"))
    sharded = jax.jit(
        shard_map(
            _body,
            mesh=mesh,
            in_specs=(PartitionSpec("core"),) * (n_params + n_outs),
            out_specs=(PartitionSpec("core"),) * n_outs,
            check_rep=False,
        ),
        keep_unused=True,
    )
    concat_in = [
        np_.concatenate([np_.asarray(in_maps[c][nm]) for c in range(n_cores)], axis=0)
        for nm in in_names
    ]
    concat_zero = [
        np_.zeros((n_cores * z.shape[0], *z.shape[1:]), z.dtype) for z in zero_outs
    ]
    staged = [jax.device_put(a, spec) for a in concat_in + concat_zero]
    jax.block_until_ready(staged)

    if profile_dir is not None:
        from antenv.axon_hooks import get_axon_ntff_profile_hook

        hook = get_axon_ntff_profile_hook()
        with hook(profile_dir, list(range(n_cores))):
            out_arrs = sharded(*staged)
            jax.block_until_ready(out_arrs)
    else:
        out_arrs = sharded(*staged)
    return [
        {
            nm: np_.asarray(out_arrs[i]).reshape(n_cores, *out_avals[i].shape)[c]
            for i, nm in enumerate(out_names)
        }
        for c in range(n_cores)
    ]


def kernel(queries, keys, values, query_weights, key_weights, value_weights):
    import sys

    for p in ("/opt/trn_rl_repo",):
        if p not in sys.path:
            sys.path.insert(0, p)

    seq, d = queries.shape
    inputs = {
        "queries": queries, "keys": keys, "values": values,
        "query_weights": query_weights, "key_weights": key_weights,
        "value_weights": value_weights,
    }
    in_maps = build_in_maps2(inputs, seq=seq, d=d)
    nc = build_nc2(seq=seq, d=d)
    results = run_spmd_staged(nc, in_maps)
    return assemble_out2(results, seq=seq)


# revision 19
# speedup vs baseline: 15.9537x; 2.5051x over previous
"""Sequence-parallel Trainium2 attention-head kernel (softmax over queries).

Shard the QUERY dim across the 8 cores (slab = 1024 queries each); every
core computes scores for ALL 8192 keys x its own query slab with j (keys)
on partitions, so the softmax-over-queries stats are per-partition-row:

  out[i,:] = sum_j exp(s_ij - M_j)/D_j * v[j,:],  M_j/D_j global over i.

Per core:
  1. Project its k/v slab (j on partitions) -> AllGather #1 (384 KB/core)
     so every core holds full projected KS=[kh;kl] and v.  Project its q
     slab -> Q1=[qh;ql], Q2=[ql;qh] (weights host-duplicated into both
     column halves so the fp32 projection lands on all 128 partitions in
     one pass; hi/lo split is then partition-aligned DVE).
  2. Per j-tile t (128 keys x 1024 local queries): 2 stacked matmuls
     (KS_t^T Q1 + KS_t^T Q2 = all 4 hi/lo cross terms), DVE negated
     row-max -> bias, ACT exp (psum->bf16) with accum_out -> local sums.
     Bias is the LOCAL max over this core's slab, so the per-row rescale
     exp(b_local - M)/D folds entirely into v later.
  3. AllGather #2 of the per-(j,core) stats [negmax|sum] (64 KB/core);
     every core reduces them to global M_j, D_j, rescales v rows by
     f_j = exp(b_local_j - M_j)/D_j, then accumulates the 64 attn
     matmuls v'_t^T e_t into psum [64 v, 1024 i] and writes its slab.
"""

import numpy as np

C = 8
QK = 64
VD = 64


def build_nc2(seq=8192, d=1024, reps=1, warmup=True, dbg=False, noscale=False):
    import concourse.bacc as bacc
    import concourse.mybir as mybir

    f32 = mybir.dt.float32
    bf16 = mybir.dt.bfloat16
    AX = mybir.AxisListType.X
    ALU = mybir.AluOpType
    ACTF = mybir.ActivationFunctionType

    NDT = d // 128            # 8 d-tiles
    SLAB = seq // C           # 1024 queries per core
    NT = seq // 128           # 64 global j-tiles
    NST = SLAB // 128         # 8 v-proj subtiles per slab
    SS = max(SLAB, 512)       # psum slot stride (bank-aligned)
    NDMA = 3 * NDT            # arena chunk DMAs per iteration (k, v, q)

    nc = bacc.Bacc(target_bir_lowering=False, debug=False, num_devices=C)

    def din(name, w, dt=bf16):
        return nc.declare_dram_parameter(name, [128, w], dt, isOutput=False)

    qin_d = din("qin", NDT * 2 * SLAB)     # col = dd*2048 + h*1024 + i
    kin_d = din("kin", NDT * 2 * SLAB)
    vin_d = din("vin", NDT * SLAB)         # col = dd*1024 + j
    wq2h_d, wq2l_d = din("wq2h", NDT * 128), din("wq2l", NDT * 128)
    wk2h_d, wk2l_d = din("wk2h", NDT * 128), din("wk2l", NDT * 128)
    wv_d = din("wv", NDT * VD)
    out_d = nc.declare_dram_parameter("out", [VD, SLAB], f32, isOutput=True)
    if dbg:
        dKS_d = nc.declare_dram_parameter("dKS", [128, seq], bf16, isOutput=True)
        dQ1_d = nc.declare_dram_parameter("dQ1", [128, SLAB], bf16, isOutput=True)
        dQ2_d = nc.declare_dram_parameter("dQ2", [128, SLAB], bf16, isOutput=True)
        dst_d = nc.declare_dram_parameter("dst", [128, 2 * NT], f32, isOutput=True)
        dsa_d = nc.declare_dram_parameter("dsa", [128, C * 2 * NT], f32, isOutput=True)
        df_d = nc.declare_dram_parameter("df", [128, NT], f32, isOutput=True)
        dvl_d = nc.declare_dram_parameter("dvl", [128, SLAB // 2], bf16, isOutput=True)
        dva_d = nc.declare_dram_parameter("dva", [128, NT * VD], bf16, isOutput=True)
        de_d = nc.declare_dram_parameter("de", [128, 4 * SLAB], bf16, isOutput=True)

    # collective bounce buffers (DRAM; collectives can't touch I/O tensors)
    cc1_in = nc.dram_tensor("cc1_in", [128, SLAB + SLAB // 2], bf16)
    cc1_out = nc.dram_tensor(
        "cc1_out", [C * 128, SLAB + SLAB // 2], bf16, addr_space="Shared"
    )
    cc2_in = nc.dram_tensor("cc2_in", [128, 2 * NT], f32)
    cc2_out = nc.dram_tensor("cc2_out", [C * 128, 2 * NT], f32, addr_space="Shared")

    from contextlib import ExitStack

    with ExitStack() as ctx:
        block = ctx.enter_context(nc.Block())
        sem = lambda n: ctx.enter_context(nc.semaphore(n))
        sb = lambda n, shape, dt: ctx.enter_context(nc.sbuf_tensor(n, shape, dt))
        ps = lambda n, shape: ctx.enter_context(nc.psum_tensor(n, shape, f32))

        s_w = sem("s_w")        # weight DMAs: 80 once
        s_in = sem("s_in")      # k/v arena DMAs: 16*16/iter (serialized)
        s_inq = sem("s_inq")    # q arena DMAs: 8*16/iter (serialized)
        s_kp = sem("s_kp")      # 8/iter (k proj per dd)
        s_vp = sem("s_vp")      # 8/iter
        s_qp = sem("s_qp")      # 8/iter
        s_ks = sem("s_ks")      # 1/iter k split done
        s_vsp = sem("s_vsp")    # 1/iter vloc copy done
        s_qs = sem("s_qs")      # 1/iter q split done
        s_gb = sem("s_gb")      # 32/iter bounce-in DMAs
        s_cc1 = sem("s_cc1")    # 1/iter
        s_ccd = sem("s_ccd")    # 256/iter gather-back DMAs
        s_sc = sem("s_sc")      # 64/iter score tiles
        s_mx = sem("s_mx")      # 64/iter
        s_ex = sem("s_ex")      # 64/iter
        s_g2 = sem("s_g2")      # 16/iter stats bounce DMA
        s_cc2 = sem("s_cc2")    # 1/iter
        s_std = sem("s_std")    # 128/iter stats gather-back
        s_sm = sem("s_sm")      # 1/iter DVE->ACT stats handoff
        s_sa = sem("s_sa")      # 1/iter ACT->DVE stats handoff
        s_vsc = sem("s_vsc")    # 64/iter v rescales
        s_at = sem("s_at")      # 1/iter attn done
        s_oc = sem("s_oc")      # 1/iter out copy done
        s_out = sem("s_out")    # 16/iter out DMA
        s_ch = sem("s_ch")      # DVE same-engine RAW chain

        arena_k = sb("arena_k", [128, 2 * 2 * SLAB], bf16)  # dd%2 ping-pong
        arena_v = sb("arena_v", [128, 2 * NDT * 128], bf16)
        arena_q = sb("arena_q", [128, 2 * 2 * SLAB], bf16)
        wq2h = sb("wq2h_s", [128, NDT * 128], bf16)
        wq2l = sb("wq2l_s", [128, NDT * 128], bf16)
        wk2h = sb("wk2h_s", [128, NDT * 128], bf16)
        wk2l = sb("wk2l_s", [128, NDT * 128], bf16)
        wv = sb("wv_s", [128, NDT * VD], bf16)
        Q1 = sb("Q1", [128, SLAB], bf16)
        Q2 = sb("Q2", [128, SLAB], bf16)
        KSloc = sb("KSloc", [128, SLAB], bf16)
        ktmp = sb("ktmp", [128, SLAB], bf16)
        vloc = sb("vloc", [128, SLAB // 2], bf16)
        KS = sb("KS", [128, seq], bf16)
        v_all = sb("v_all", [128, NT * VD], bf16)
        e_sb = sb("e_sb", [128, NT * SLAB], bf16)           # 128 KB/part
        stats_loc = sb("stats_loc", [128, 2 * NT], f32)     # [negmax | sum]
        stats_all = sb("stats_all", [128, C * 2 * NT], f32)
        NM = sb("NM", [128, NT], f32)
        darg = sb("darg", [128, C * NT], f32)
        w8 = sb("w8", [128, C * NT], f32)
        wD = sb("wD", [128, C * NT], f32)
        Dt = sb("Dt", [128, NT], f32)
        Rt = sb("Rt", [128, NT], f32)
        wlarg = sb("wlarg", [128, NT], f32)
        wl = sb("wl", [128, NT], f32)
        f_sb = sb("f_sb", [128, NT], f32)

        out_sb = sb("out_sb", [VD, SLAB], f32)
        ps_s = ps("ps_s", [128, 3 * SS])     # proj + score slots (6 banks)
        ps_o = ps("ps_o", [VD, SLAB])        # 2 banks
        ps_v = ps_s[:, 2 * SS : 2 * SS + SLAB // 2]   # v proj in slot 2

        # ---------------- SYNC: k/v input + output DMAs ----------------
        # per-chunk s_in levels must be ordered sync points, so each arena
        # DMA is serialized (waited) before the next is issued.
        @block.sync
        def _(s):
          for it in range(reps):
            n_in = it * (NDT + NST)
            for dd in range(NDT):          # k chunks
                g = it * NDT + dd
                if g >= 2:
                    s.wait_ge(s_kp, g - 1)
                s.dma_start(
                    out=arena_k[:, (dd % 2) * 2 * SLAB : (dd % 2 + 1) * 2 * SLAB],
                    in_=kin_d[:, dd * 2 * SLAB : (dd + 1) * 2 * SLAB],
                ).then_inc(s_in, 16)
                n_in += 1
                s.wait_ge(s_in, n_in * 16)
            for st in range(NST):          # v chunks (one j-subtile, all dd)
                g = it * NST + st
                if g >= 2:
                    s.wait_ge(s_vp, g - 1)
                s.dma_start(
                    out=arena_v[:, (st % 2) * NDT * 128 : (st % 2 + 1) * NDT * 128],
                    in_=vin_d[:, st * NDT * 128 : (st + 1) * NDT * 128],
                ).then_inc(s_in, 16)
                n_in += 1
                s.wait_ge(s_in, n_in * 16)
            s.wait_ge(s_oc, it + 1)
            s.dma_start(out=out_d[:, :], in_=out_sb[:, :]).then_inc(s_out, 16)
            s.wait_ge(s_out, it * 16 + 16)
            if dbg and it == reps - 1:
                n_d = 0
                for dst, srcb in ((dKS_d, KS), (dQ1_d, Q1), (dQ2_d, Q2),
                                  (dst_d, stats_loc), (dsa_d, stats_all),
                                  (df_d, f_sb), (dva_d, v_all),
                                  (dvl_d, vloc)):
                    s.dma_start(out=dst[:, :], in_=srcb[:, :]).then_inc(s_out, 16)
                    n_d += 1
                s.dma_start(
                    out=de_d[:, :], in_=e_sb[:, 0 : 4 * SLAB]
                ).then_inc(s_out, 16)
                n_d += 1
                s.wait_ge(s_out, it * 16 + 16 + n_d * 16)

        # ---------------- TENSOR (PE) ----------------
        @block.tensor
        def _(t):
          for it in range(reps):
            for w_ in range(40 if (warmup and it == 0) else 0):
                t.matmul(
                    ps_s[0:64, 0:512], Q1[:, 0:64], Q1[:, 0 : min(SLAB, 512)],
                    start=(w_ == 0), stop=False,
                )
            if it == 0:
                t.wait_ge(s_w, 80)
            # ---- k proj -> ps_s[:, 0:SLAB] (both halves via dup weights)
            if it > 0:
                t.wait_ge(s_ex, it * NT)       # all prev-rep slots free
            for dd in range(NDT):
                t.wait_ge(s_in, (it * (NDT + NST) + dd + 1) * 16)
                kb = (dd % 2) * 2 * SLAB
                uh = arena_k[:, kb : kb + SLAB]
                ul = arena_k[:, kb + SLAB : kb + 2 * SLAB]
                for ti, (W, X) in enumerate(((wk2h, uh), (wk2h, ul), (wk2l, uh))):
                    for hb in range(0, SLAB, 512):
                        he = min(hb + 512, SLAB)
                        mm = t.matmul(
                            ps_s[:, hb:he],
                            W[:, dd * 128 : (dd + 1) * 128], X[:, hb:he],
                            start=(dd == 0 and ti == 0),
                            stop=(dd == NDT - 1 and ti == 2),
                            skip_group_check=True,
                        )
                mm.then_inc(s_kp, 1)
            # ---- v proj -> ps_v (st-outer: one sequential group per
            # subtile; vin is re-tiled so chunk st holds all dd for its j's)
            if it > 0:
                t.wait_ge(s_vsp, it)           # ps_v free
            for st in range(NST):
                t.wait_ge(s_in, (it * (NDT + NST) + NDT + st + 1) * 16)
                vb = (st % 2) * NDT * 128
                for dd in range(NDT):
                    mm = t.matmul(
                        ps_v[:, st * VD : (st + 1) * VD],
                        arena_v[:, vb + dd * 128 : vb + (dd + 1) * 128],
                        wv[:, dd * VD : (dd + 1) * VD],
                        start=(dd == 0), stop=(dd == NDT - 1),
                    )
                mm.then_inc(s_vp, 1)
            # ---- q proj -> ps_s[:, SLAB:2*SLAB]
            if it > 0:
                t.wait_ge(s_ex, it * NT)       # slot1 free (prev tile 63)
            for dd in range(NDT):
                t.wait_ge(s_inq, (it * NDT + dd + 1) * 16)
                qb = (dd % 2) * 2 * SLAB
                uh = arena_q[:, qb : qb + SLAB]
                ul = arena_q[:, qb + SLAB : qb + 2 * SLAB]
                for ti, (W, X) in enumerate(((wq2h, uh), (wq2h, ul), (wq2l, uh))):
                    for hb in range(0, SLAB, 512):
                        he = min(hb + 512, SLAB)
                        mm = t.matmul(
                            ps_s[:, SS + hb : SS + he],
                            W[:, dd * 128 : (dd + 1) * 128], X[:, hb:he],
                            start=(dd == 0 and ti == 0),
                            stop=(dd == NDT - 1 and ti == 2),
                            skip_group_check=True,
                        )
                mm.then_inc(s_qp, 1)
            # ---- scores: 64 j-tiles
            t.wait_ge(s_ccd, (it + 1) * 256)
            t.wait_ge(s_qs, it + 1)
            for tt in range(NT):
                g = it * NT + tt
                if g >= 3:
                    t.wait_ge(s_ex, g - 2)     # slot free (exp of tt-3)
                so = (tt % 3) * SS
                kt = KS[:, tt * 128 : (tt + 1) * 128]
                for hb in range(0, SLAB, 512):
                    he = min(hb + 512, SLAB)
                    t.matmul(
                        ps_s[:, so + hb : so + he], kt, Q1[:, hb:he],
                        start=True, stop=False, skip_group_check=True,
                    )
                    mm = t.matmul(
                        ps_s[:, so + hb : so + he], kt, Q2[:, hb:he],
                        start=False, stop=True, skip_group_check=True,
                    )
                mm.then_inc(s_sc, 1)
            # ---- attn: accumulate 64 tiles into ps_o
            t.wait_ge(s_oc, it)                # ps_o free
            for tt in range(NT):
                t.wait_ge(s_vsc, it * NT + tt + 1)
                for hb in range(0, SLAB, 512):
                    he = min(hb + 512, SLAB)
                    mm = t.matmul(
                        ps_o[:, hb:he],
                        v_all[:, tt * VD : (tt + 1) * VD],
                        e_sb[:, tt * SLAB + hb : tt * SLAB + he],
                        start=(tt == 0), stop=(tt == NT - 1),
                        skip_group_check=True,
                    )
            mm.then_inc(s_at, 1)

        # ---------------- VECTOR (DVE) ----------------
        ch_n = [0]

        @block.vector
        def _(v):
          def step(inst):
              inst.then_inc(s_ch, 1)
              ch_n[0] += 1

          def cw(v):
              v.wait_ge(s_ch, ch_n[0])

          for it in range(reps):
            # k split -> KSloc = [kh; kl]
            v.wait_ge(s_kp, it * NDT + NDT)
            if it > 0:
                v.wait_ge(s_gb, it * 32)       # KSloc/vloc consumed
            step(v.tensor_copy(KSloc[0:64, :], ps_s[0:64, 0:SLAB]))
            step(v.tensor_copy(ktmp[64:128, :], ps_s[64:128, 0:SLAB]))
            cw(v)
            v.tensor_tensor(
                KSloc[64:128, :], ps_s[64:128, 0:SLAB], ktmp[64:128, :],
                op=ALU.subtract,
            ).then_inc(s_ks, 1)
            # v copy
            v.wait_ge(s_vp, it * NST + NST)
            v.tensor_copy(vloc[:, :], ps_v[:, :]).then_inc(s_vsp, 1)
            # q split -> Q1=[qh;ql], Q2=[ql;qh]
            v.wait_ge(s_qp, it * NDT + NDT)
            qsl = ps_s[:, SS : SS + SLAB]
            step(v.tensor_copy(Q1[0:64, :], qsl[0:64, :]))
            step(v.tensor_copy(Q2[64:128, :], qsl[64:128, :]))
            cw(v)
            v.tensor_tensor(
                Q1[64:128, :], qsl[64:128, :], Q2[64:128, :], op=ALU.subtract
            )
            v.tensor_tensor(
                Q2[0:64, :], qsl[0:64, :], Q1[0:64, :], op=ALU.subtract
            ).then_inc(s_qs, 1)
            # per-tile negated row-max
            for tt in range(NT):
                v.wait_ge(s_sc, it * NT + tt + 1)
                if tt == 0 and it > 0:
                    v.wait_ge(s_g2, it * 16)   # stats_loc consumed
                v.tensor_reduce(
                    stats_loc[:, tt : tt + 1],
                    ps_s[:, (tt % 3) * SS : (tt % 3) * SS + SLAB],
                    axis=AX, op=ALU.max, negate=True,
                ).then_inc(s_mx, 1)
            # stats math
            v.wait_ge(s_std, (it + 1) * 128)
            nb_v = stats_all[:, :].rearrange("p (c t) -> p t c", c=C, t=2 * NT)
            step(v.tensor_reduce(NM[:, :], nb_v[:, 0:NT, :], axis=AX, op=ALU.min))
            cw(v)
            for c in range(C):
                v.tensor_tensor(
                    darg[:, c * NT : (c + 1) * NT], NM[:, :],
                    stats_all[:, c * 2 * NT : c * 2 * NT + NT],
                    op=ALU.subtract,
                )
            v.tensor_tensor(
                wlarg[:, :], NM[:, :], stats_loc[:, 0:NT], op=ALU.subtract
            ).then_inc(s_sm, 1)
            v.wait_ge(s_sa, it + 1)
            dl_v = stats_all[:, :].rearrange("p (c t) -> p c t", c=C, t=2 * NT)
            step(v.tensor_tensor(
                wD[:, :].rearrange("p (c t) -> p c t", c=C, t=NT),
                w8[:, :].rearrange("p (c t) -> p c t", c=C, t=NT),
                dl_v[:, :, NT : 2 * NT],
                op=ALU.mult,
            ))
            cw(v)
            step(v.tensor_reduce(
                Dt[:, :],
                wD[:, :].rearrange("p (c t) -> p t c", c=C, t=NT),
                axis=AX, op=ALU.add,
            ))
            cw(v)
            step(v.reciprocal(Rt[:, :], Dt[:, :]))
            cw(v)
            step(v.tensor_tensor(f_sb[:, :], wl[:, :], Rt[:, :], op=ALU.mult))
            cw(v)
            # rescale v rows (in place)
            v.wait_ge(s_ccd, (it + 1) * 256)
            for tt in range(NT):
                v.tensor_scalar_mul(
                    v_all[:, tt * VD : (tt + 1) * VD],
                    v_all[:, tt * VD : (tt + 1) * VD],
                    1.0 if noscale else f_sb[:, tt : tt + 1],
                ).then_inc(s_vsc, 1)

        # ---------------- SCALAR (ACT) ----------------
        @block.scalar
        def _(sc):
          for it in range(reps):
            # q arena stream on the ACT queue (serialized per-chunk levels)
            for dd in range(NDT):
                g = it * NDT + dd
                if g >= 2:
                    sc.wait_ge(s_qp, g - 1)
                sc.dma_start(
                    out=arena_q[:, (dd % 2) * 2 * SLAB : (dd % 2 + 1) * 2 * SLAB],
                    in_=qin_d[:, dd * 2 * SLAB : (dd + 1) * 2 * SLAB],
                ).then_inc(s_inq, 16)
                sc.wait_ge(s_inq, (it * NDT + dd + 1) * 16)
            for tt in range(NT):
                sc.wait_ge(s_mx, it * NT + tt + 1)
                if tt == 0 and it > 0:
                    sc.wait_ge(s_at, it)       # e_sb consumed by attn
                sc.activation(
                    e_sb[:, tt * SLAB : (tt + 1) * SLAB],
                    ps_s[:, (tt % 3) * SS : (tt % 3) * SS + SLAB],
                    ACTF.Exp,
                    bias=stats_loc[:, tt : tt + 1], scale=1.0,
                    accum_out=stats_loc[:, NT + tt : NT + tt + 1],
                ).then_inc(s_ex, 1)
            sc.wait_ge(s_sm, it + 1)
            sc.activation(w8[:, :], darg[:, :], ACTF.Exp)
            sc.activation(wl[:, :], wlarg[:, :], ACTF.Exp).then_inc(s_sa, 1)
            sc.wait_ge(s_at, it + 1)
            if it > 0:
                sc.wait_ge(s_out, it * 16)     # out_sb consumed
            sc.activation(out_sb[:, :], ps_o[:, :], ACTF.Copy).then_inc(s_oc, 1)

        # ---------------- GPSIMD: collectives ----------------
        @block.gpsimd
        def _(g):
          for it in range(reps):
            if it == 0:
                for wsb, wdr in ((wq2h, wq2h_d), (wq2l, wq2l_d),
                                 (wk2h, wk2h_d), (wk2l, wk2l_d), (wv, wv_d)):
                    g.dma_start(out=wsb[:, :], in_=wdr[:, :]).then_inc(s_w, 16)
            g.wait_ge(s_ks, it + 1)
            g.wait_ge(s_vsp, it + 1)
            if it > 0:
                g.wait_ge(s_cc1, it)           # cc1_in consumed
            g.dma_start(out=cc1_in[:, 0:SLAB], in_=KSloc[:, :]).then_inc(s_gb, 16)
            g.dma_start(
                out=cc1_in[:, SLAB : SLAB + SLAB // 2], in_=vloc[:, :]
            ).then_inc(s_gb, 16)
            g.wait_ge(s_gb, it * 32 + 32)
            g.collective_compute(
                "AllGather", mybir.AluOpType.bypass,
                replica_groups=[list(range(C))],
                ins=[cc1_in[:, :].opt()],
                outs=[cc1_out[:, :].opt()],
            ).then_inc(s_cc1, 1)
            g.wait_ge(s_cc1, it + 1)
            for c in range(C):
                g.dma_start(
                    out=KS[:, c * SLAB : (c + 1) * SLAB],
                    in_=cc1_out[c * 128 : (c + 1) * 128, 0:SLAB],
                ).then_inc(s_ccd, 16)
            for c in range(C):
                g.dma_start(
                    out=v_all[:, c * (SLAB // 2) : (c + 1) * (SLAB // 2)],
                    in_=cc1_out[c * 128 : (c + 1) * 128, SLAB : SLAB + SLAB // 2],
                ).then_inc(s_ccd, 16)
            # stats gather
            g.wait_ge(s_ex, (it + 1) * NT)
            if it > 0:
                g.wait_ge(s_cc2, it)
            g.dma_start(out=cc2_in[:, :], in_=stats_loc[:, :]).then_inc(s_g2, 16)
            g.wait_ge(s_g2, (it + 1) * 16)
            g.collective_compute(
                "AllGather", mybir.AluOpType.bypass,
                replica_groups=[list(range(C))],
                ins=[cc2_in[:, :].opt()],
                outs=[cc2_out[:, :].opt()],
            ).then_inc(s_cc2, 1)
            g.wait_ge(s_cc2, it + 1)
            for c in range(C):
                g.dma_start(
                    out=stats_all[:, c * 2 * NT : (c + 1) * 2 * NT],
                    in_=cc2_out[c * 128 : (c + 1) * 128, :],
                ).then_inc(s_std, 16)

    nc.finalize()
    return nc


# ------------------------- host side -------------------------

def _split_bf16(x):
    import ml_dtypes

    hi = x.astype(ml_dtypes.bfloat16)
    lo = (x - hi.astype(np.float32)).astype(ml_dtypes.bfloat16)
    return hi, lo


def _tile_cols(xT, w):
    """[d, s] -> [128, (d/128)*w] with col = dd*w + i (s == w per d-tile)."""
    dd = xT.shape[0] // 128
    return np.ascontiguousarray(
        xT.reshape(dd, 128, w).transpose(1, 0, 2).reshape(128, dd * w)
    )


def build_in_maps2(inputs, seq=8192, d=1024):
    import ml_dtypes

    bf = ml_dtypes.bfloat16
    SLAB = seq // C
    NDT = d // 128

    qw = (inputs["query_weights"] / np.sqrt(np.float32(QK))).astype(np.float32)
    wqh, wql = _split_bf16(qw)
    wkh, wkl = _split_bf16(inputs["key_weights"].astype(np.float32))

    def dup_tile(w):
        w2 = np.concatenate([w.astype(np.float32)] * 2, axis=1)  # [d, 128]
        return _tile_cols(w2, 128).astype(bf)

    w_maps = {
        "wq2h": dup_tile(wqh), "wq2l": dup_tile(wql),
        "wk2h": dup_tile(wkh), "wk2l": dup_tile(wkl),
        "wv": _tile_cols(
            inputs["value_weights"].astype(np.float32), VD
        ).astype(bf),
    }

    def slab_hi_lo(xT):
        """[d, SLAB] -> [128, NDT*2*SLAB] with col = dd*2*SLAB + h*SLAB + i."""
        hi, lo = _split_bf16(xT)
        ht = _tile_cols(hi.astype(np.float32), SLAB).reshape(128, NDT, SLAB)
        lt = _tile_cols(lo.astype(np.float32), SLAB).reshape(128, NDT, SLAB)
        return np.ascontiguousarray(
            np.concatenate([ht[:, :, None, :], lt[:, :, None, :]], axis=2)
            .reshape(128, NDT * 2 * SLAB)
        ).astype(bf)

    def _vin_tile(xT):
        """[d, SLAB] -> [128, NST*NDT*128], col = st*NDT*128 + dd*128 + jj."""
        t = _tile_cols(xT, SLAB).reshape(128, NDT, SLAB // 128, 128)
        return np.ascontiguousarray(
            t.transpose(0, 2, 1, 3).reshape(128, -1)
        )

    qT = np.ascontiguousarray(inputs["queries"].T).astype(np.float32)
    kT = np.ascontiguousarray(inputs["keys"].T).astype(np.float32)
    vT = np.ascontiguousarray(inputs["values"].T).astype(np.float32)

    in_maps = []
    for c in range(C):
        sl = slice(c * SLAB, (c + 1) * SLAB)
        m = {
            "qin": slab_hi_lo(qT[:, sl]),
            "kin": slab_hi_lo(kT[:, sl]),
            "vin": _vin_tile(vT[:, sl]).astype(bf),
        }
        m.update(w_maps)
        in_maps.append(m)
    return in_maps


def assemble_out2(results, seq=8192):
    SLAB = seq // C
    full = np.zeros((seq, VD), np.float32)
    for c in range(C):
        o = np.asarray(results[c]["out"], dtype=np.float32)  # [VD, SLAB]
        full[c * SLAB : (c + 1) * SLAB] = o.T
    return full


def run_spmd_staged(nc, in_maps, profile_dir=None):
    """run_bass_via_pjrt with inputs pre-staged on-device (blocks until all
    shards are resident) so the 8 cores launch aligned instead of staggered
    by per-device input-transfer time."""
    import jax
    import numpy as np_
    from jax.sharding import Mesh, PartitionSpec, NamedSharding
    from jax.experimental.shard_map import shard_map
    import concourse.mybir as mybir
    from concourse import bass2jax

    bass2jax.install_neuronx_cc_hook()
    n_cores = len(in_maps)

    partition_name = (
        nc.partition_id_tensor.name if nc.partition_id_tensor else None
    )
    in_names, out_names, out_avals, zero_outs = [], [], [], []
    for alloc in nc.m.functions[0].allocations:
        if not isinstance(alloc, mybir.MemoryLocationSet):
            continue
        name = alloc.memorylocations[0].name
        if alloc.kind == "ExternalInput":
            if name != partition_name:
                in_names.append(name)
        elif alloc.kind == "ExternalOutput":
            out_names.append(name)
            shape = tuple(alloc.tensor_shape)
            dtype = mybir.dt.np(alloc.dtype)
            out_avals.append(jax.core.ShapedArray(shape, dtype))
            zero_outs.append(np_.zeros(shape, dtype))
    n_params = len(in_names)
    n_outs = len(out_avals)
    all_names = in_names + out_names
    if partition_name is not None:
        all_names = all_names + [partition_name]

    def _body(*args):
        operands = list(args)
        if partition_name is not None:
            operands.append(bass2jax.partition_id_tensor())
        outs = bass2jax._bass_exec_p.bind(
            *operands,
            out_avals=tuple(out_avals),
            in_names=tuple(all_names),
            out_names=tuple(out_names),
            lowering_input_output_aliases=(),
            sim_require_finite=True,
            sim_require_nnan=True,
            nc=nc,
        )
        return tuple(outs)

    devices = jax.devices()[:n_cores]
    mesh = Mesh(np_.asarray(devices), ("core",))
    spec = NamedSharding(mesh, PartitionSpec("core"))
    sharded = jax.jit(
        shard_map(
            _body,
            mesh=mesh,
            in_specs=(PartitionSpec("core"),) * (n_params + n_outs),
            out_specs=(PartitionSpec("core"),) * n_outs,
            check_rep=False,
        ),
        keep_unused=True,
    )
    concat_in = [
        np_.concatenate([np_.asarray(in_maps[c][nm]) for c in range(n_cores)], axis=0)
        for nm in in_names
    ]
    concat_zero = [
        np_.zeros((n_cores * z.shape[0], *z.shape[1:]), z.dtype) for z in zero_outs
    ]
    staged = [jax.device_put(a, spec) for a in concat_in + concat_zero]
    jax.block_until_ready(staged)

    if profile_dir is not None:
        from antenv.axon_hooks import get_axon_ntff_profile_hook

        hook = get_axon_ntff_profile_hook()
        with hook(profile_dir, list(range(n_cores))):
            out_arrs = sharded(*staged)
            jax.block_until_ready(out_arrs)
    else:
        out_arrs = sharded(*staged)
    return [
        {
            nm: np_.asarray(out_arrs[i]).reshape(n_cores, *out_avals[i].shape)[c]
            for i, nm in enumerate(out_names)
        }
        for c in range(n_cores)
    ]


def kernel(queries, keys, values, query_weights, key_weights, value_weights):
    import sys

    for p in ("/opt/trn_rl_repo",):
        if p not in sys.path:
            sys.path.insert(0, p)

    seq, d = queries.shape
    inputs = {
        "queries": queries, "keys": keys, "values": values,
        "query_weights": query_weights, "key_weights": key_weights,
        "value_weights": value_weights,
    }
    in_maps = build_in_maps2(inputs, seq=seq, d=d)
    nc = build_nc2(seq=seq, d=d)
    results = run_spmd_staged(nc, in_maps)
    return assemble_out2(results, seq=seq)


# revision 29
# speedup vs baseline: 53.6171x; 3.3608x over previous
"""Sequence-parallel Trainium2 attention-head kernel (softmax over queries).

Shard the QUERY dim across the 8 cores (slab = 1024 queries each); every
core computes scores for ALL 8192 keys x its own query slab with j (keys)
on partitions, so the softmax-over-queries stats are per-partition-row:

  out[i,:] = sum_j exp(s_ij - M_j)/D_j * v[j,:],  M_j/D_j global over i.

Per core:
  1. Project its k/v slab (j on partitions) -> AllGather #1 (384 KB/core)
     so every core holds full projected KS=[kh;kl] and v.  Project its q
     slab -> Q1=[qh;ql], Q2=[ql;qh] (weights host-duplicated into both
     column halves so the fp32 projection lands on all 128 partitions in
     one pass; hi/lo split is then partition-aligned DVE).
  2. Per j-tile t (128 keys x 1024 local queries): 2 stacked matmuls
     (KS_t^T Q1 + KS_t^T Q2 = all 4 hi/lo cross terms), DVE negated
     row-max -> bias, ACT exp (psum->bf16) with accum_out -> local sums.
     Bias is the LOCAL max over this core's slab, so the per-row rescale
     exp(b_local - M)/D folds entirely into v later.
  3. AllGather #2 of the per-(j,core) stats [negmax|sum] (64 KB/core);
     every core reduces them to global M_j, D_j, rescales v rows by
     f_j = exp(b_local_j - M_j)/D_j, then accumulates the 64 attn
     matmuls v'_t^T e_t into psum [64 v, 1024 i] and writes its slab.
"""

import numpy as np

C = 8
QK = 64
VD = 64


def build_nc2(seq=8192, d=1024, reps=1, warmup=True, dbg=False, noscale=False, attn_lite=False, scores_lite=False, no_cc=False):
    import concourse.bacc as bacc
    import concourse.mybir as mybir

    f32 = mybir.dt.float32
    bf16 = mybir.dt.bfloat16
    AX = mybir.AxisListType.X
    ALU = mybir.AluOpType
    ACTF = mybir.ActivationFunctionType

    NDT = d // 128            # 8 d-tiles
    SLAB = seq // C           # 1024 queries per core
    NT = seq // 128           # 64 global j-tiles
    NST = SLAB // 128         # 8 v-proj subtiles per slab
    SS = max(SLAB, 512)       # psum slot stride (bank-aligned)
    NDMA = 3 * NDT            # arena chunk DMAs per iteration (k, v, q)
    INF = 3.0e38

    nc = bacc.Bacc(target_bir_lowering=False, debug=False, num_devices=C)

    def din(name, w, dt=bf16):
        return nc.declare_dram_parameter(name, [128, w], dt, isOutput=False)

    qin_d = din("qin", NDT * 2 * SLAB)     # col = dd*2048 + h*1024 + i
    kin_d = din("kin", NDT * 2 * SLAB)
    vin_d = din("vin", NDT * SLAB)         # col = dd*1024 + j
    wq2h_d, wq2l_d = din("wq2h", NDT * 128), din("wq2l", NDT * 128)
    wk2h_d, wk2l_d = din("wk2h", NDT * 128), din("wk2l", NDT * 128)
    wv_d = din("wv", NDT * VD)
    out_d = nc.declare_dram_parameter("out", [VD, SLAB], f32, isOutput=True)
    if dbg:
        dKS_d = nc.declare_dram_parameter("dKS", [128, seq], bf16, isOutput=True)
        dQ1_d = nc.declare_dram_parameter("dQ1", [128, SLAB], bf16, isOutput=True)
        dQ2_d = nc.declare_dram_parameter("dQ2", [128, SLAB], bf16, isOutput=True)
        dst_d = nc.declare_dram_parameter("dst", [128, 2 * NT], f32, isOutput=True)
        dsa_d = nc.declare_dram_parameter("dsa", [128, C * 2 * NT], f32, isOutput=True)
        df_d = nc.declare_dram_parameter("df", [128, NT], f32, isOutput=True)
        dvl_d = nc.declare_dram_parameter("dvl", [128, SLAB // 2], bf16, isOutput=True)
        dva_d = nc.declare_dram_parameter("dva", [128, NT * VD], bf16, isOutput=True)
        de_d = nc.declare_dram_parameter("de", [128, 4 * SLAB], bf16, isOutput=True)

    # collective bounce buffers (DRAM; collectives can't touch I/O tensors)
    NKG = min(4, SLAB // 128) # KS gather chunks (pipelined collectives)
    KGW = SLAB // NKG         # j-columns per chunk
    cc1_ins = [
        nc.dram_tensor(f"cc1i_{g}", [128, KGW], bf16) for g in range(NKG)
    ]
    ccv_in = nc.dram_tensor("ccv_in", [128, SLAB // 2], bf16)
    cc1_outs = [
        nc.dram_tensor(f"cc1o_{g}", [C * 128, KGW], bf16, addr_space="Shared")
        for g in range(NKG)
    ]
    ccv_out = nc.dram_tensor(
        "ccv_out", [C * 128, SLAB // 2], bf16, addr_space="Shared"
    )
    cc2_in = nc.dram_tensor("cc2_in", [128, 2 * NT], f32)
    cc2_out = nc.dram_tensor("cc2_out", [C * 128, 2 * NT], f32, addr_space="Shared")

    from contextlib import ExitStack

    with ExitStack() as ctx:
        block = ctx.enter_context(nc.Block())
        sem = lambda n: ctx.enter_context(nc.semaphore(n))
        sb = lambda n, shape, dt: ctx.enter_context(nc.sbuf_tensor(n, shape, dt))
        ps = lambda n, shape: ctx.enter_context(nc.psum_tensor(n, shape, f32))

        s_w = sem("s_w")        # weight DMAs: 80 once
        s_in = sem("s_in")      # k/v arena DMAs: 16*16/iter (serialized)
        s_inq = sem("s_inq")    # q arena DMAs: 8*16/iter (serialized)
        s_kp = sem("s_kp")      # 8/iter (k proj per dd)
        s_vp = sem("s_vp")      # 8/iter
        s_qp = sem("s_qp")      # 8/iter
        s_ks = sem("s_ks")      # 1/iter k split done
        s_vsp = sem("s_vsp")    # 1/iter vloc copy done
        s_qs = sem("s_qs")      # 1/iter q split done
        s_gb = sem("s_gb")      # 32/iter bounce-in DMAs
        s_cc1 = sem("s_cc1")    # 1/iter
        s_ccd = sem("s_ccd")    # 256/iter gather-back DMAs
        s_sc = sem("s_sc")      # 64/iter score tiles
        s_mx = sem("s_mx")      # 64/iter
        s_ex = sem("s_ex")      # 64/iter
        s_g2 = sem("s_g2")      # 16/iter stats bounce DMA
        s_cc2 = sem("s_cc2")    # 1/iter
        s_std = sem("s_std")    # 128/iter stats gather-back
        s_sm = sem("s_sm")      # 1/iter DVE->ACT stats handoff
        s_sa = sem("s_sa")      # 1/iter ACT->DVE stats handoff
        s_vsc = sem("s_vsc")    # 64/iter v rescales
        s_at = sem("s_at")      # 1/iter attn done
        s_oc = sem("s_oc")      # 1/iter out copy done
        s_out = sem("s_out")    # 16/iter out DMA
        s_ch = sem("s_ch")      # DVE same-engine RAW chain

        arena_k = sb("arena_k", [128, 2 * 2 * SLAB], bf16)  # dd%2 ping-pong
        arena_v = sb("arena_v", [128, 2 * NDT * 128], bf16)
        arena_q = sb("arena_q", [128, 2 * 2 * SLAB], bf16)
        wq2h = sb("wq2h_s", [128, NDT * 128], bf16)
        wq2l = sb("wq2l_s", [128, NDT * 128], bf16)
        wk2h = sb("wk2h_s", [128, NDT * 128], bf16)
        wk2l = sb("wk2l_s", [128, NDT * 128], bf16)
        wv = sb("wv_s", [128, NDT * VD], bf16)
        Q1 = sb("Q1", [128, SLAB], bf16)
        Q2 = sb("Q2", [128, SLAB], bf16)
        KSloc = sb("KSloc", [128, SLAB], bf16)
        ktmp = sb("ktmp", [128, SLAB], bf16)
        vloc = sb("vloc", [128, SLAB // 2], bf16)
        KS = sb("KS", [128, seq], bf16)
        v_all = sb("v_all", [128, NT * VD], bf16)
        e_sb = sb("e_sb", [128, NT * SLAB], bf16)           # 128 KB/part
        stats_loc = sb("stats_loc", [128, 2 * NT], f32)     # [negmax | sum]
        stats_all = sb("stats_all", [128, C * 2 * NT], f32)
        NM = sb("NM", [128, NT], f32)
        darg = sb("darg", [128, C * NT], f32)
        w8 = sb("w8", [128, C * NT], f32)
        wD = sb("wD", [128, C * NT], f32)
        Dt = sb("Dt", [128, NT], f32)
        Rt = sb("Rt", [128, NT], f32)
        wlarg = sb("wlarg", [128, NT], f32)
        wl = sb("wl", [128, NT], f32)
        f_sb = sb("f_sb", [128, NT], f32)

        out_sb = sb("out_sb", [VD, SLAB], f32)
        ps_s = ps("ps_s", [128, 3 * SS])     # proj + score slots (6 banks)
        ps_o = ps("ps_o", [VD, SLAB])        # 2 banks
        ps_v = ps_s[:, 2 * SS : 2 * SS + SLAB // 2]   # v proj in slot 2

        TORD = [
            c * (SLAB // 128) + gi * (KGW // 128) + u
            for gi in range(NKG)
            for c in range(C)
            for u in range(KGW // 128)
        ]

        # ---------------- SYNC: k/v input + output DMAs ----------------
        # per-chunk s_in levels must be ordered sync points, so each arena
        # DMA is serialized (waited) before the next is issued.
        @block.sync
        def _(s):
          for it in range(reps):
            n_in = it * (NDT + NST)
            for dd in range(NDT):          # k chunks
                g = it * NDT + dd
                if g >= 2:
                    s.wait_ge(s_kp, g - 1)
                s.dma_start(
                    out=arena_k[:, (dd % 2) * 2 * SLAB : (dd % 2 + 1) * 2 * SLAB],
                    in_=kin_d[:, dd * 2 * SLAB : (dd + 1) * 2 * SLAB],
                ).then_inc(s_in, 16)
                n_in += 1
                s.wait_ge(s_in, n_in * 16)
            for st in range(NST):          # v chunks (one j-subtile, all dd)
                g = it * NST + st
                if g >= 2:
                    s.wait_ge(s_vp, g - 1)
                s.dma_start(
                    out=arena_v[:, (st % 2) * NDT * 128 : (st % 2 + 1) * NDT * 128],
                    in_=vin_d[:, st * NDT * 128 : (st + 1) * NDT * 128],
                ).then_inc(s_in, 16)
                n_in += 1
                s.wait_ge(s_in, n_in * 16)
            s.wait_ge(s_oc, it + 1)
            s.dma_start(out=out_d[:, :], in_=out_sb[:, :]).then_inc(s_out, 16)
            s.wait_ge(s_out, it * 16 + 16)
            if dbg and it == reps - 1:
                n_d = 0
                for dst, srcb in ((dKS_d, KS), (dQ1_d, Q1), (dQ2_d, Q2),
                                  (dst_d, stats_loc), (dsa_d, stats_all),
                                  (df_d, f_sb), (dva_d, v_all),
                                  (dvl_d, vloc)):
                    s.dma_start(out=dst[:, :], in_=srcb[:, :]).then_inc(s_out, 16)
                    n_d += 1
                s.dma_start(
                    out=de_d[:, :], in_=e_sb[:, 0 : 4 * SLAB]
                ).then_inc(s_out, 16)
                n_d += 1
                s.wait_ge(s_out, it * 16 + 16 + n_d * 16)

        # ---------------- TENSOR (PE) ----------------
        @block.tensor
        def _(t):
          for it in range(reps):
            for w_ in range(40 if (warmup and it == 0) else 0):
                t.matmul(
                    ps_s[0:64, 0:512], Q1[:, 0:64], Q1[:, 0 : min(SLAB, 512)],
                    start=(w_ == 0), stop=False,
                )
            if it == 0:
                t.wait_ge(s_w, 80)
            # ---- k proj -> ps_s[:, 0:SLAB] (both halves via dup weights)
            if it > 0:
                t.wait_ge(s_ex, it * NT)       # all prev-rep slots free
            for dd in range(NDT):
                t.wait_ge(s_in, (it * (NDT + NST) + dd + 1) * 16)
                kb = (dd % 2) * 2 * SLAB
                uh = arena_k[:, kb : kb + SLAB]
                ul = arena_k[:, kb + SLAB : kb + 2 * SLAB]
                for ti, (W, X) in enumerate(((wk2h, uh), (wk2h, ul), (wk2l, uh))):
                    for hb in range(0, SLAB, 512):
                        he = min(hb + 512, SLAB)
                        mm = t.matmul(
                            ps_s[:, hb:he],
                            W[:, dd * 128 : (dd + 1) * 128], X[:, hb:he],
                            start=(dd == 0 and ti == 0),
                            stop=(dd == NDT - 1 and ti == 2),
                            skip_group_check=True,
                        )
                mm.then_inc(s_kp, 1)
            # ---- v proj -> ps_v (st-outer: one sequential group per
            # subtile; vin is re-tiled so chunk st holds all dd for its j's)
            if it > 0:
                t.wait_ge(s_vsp, it)           # ps_v free
            for st in range(NST):
                t.wait_ge(s_in, (it * (NDT + NST) + NDT + st + 1) * 16)
                vb = (st % 2) * NDT * 128
                for dd in range(NDT):
                    mm = t.matmul(
                        ps_v[:, st * VD : (st + 1) * VD],
                        arena_v[:, vb + dd * 128 : vb + (dd + 1) * 128],
                        wv[:, dd * VD : (dd + 1) * VD],
                        start=(dd == 0), stop=(dd == NDT - 1),
                    )
                mm.then_inc(s_vp, 1)
            # ---- q proj -> ps_s[:, SLAB:2*SLAB]
            if it > 0:
                t.wait_ge(s_ex, it * NT)       # slot1 free (prev tile 63)
            for dd in range(NDT):
                t.wait_ge(s_inq, (it * NDT + dd + 1) * 16)
                qb = (dd % 2) * 2 * SLAB
                uh = arena_q[:, qb : qb + SLAB]
                ul = arena_q[:, qb + SLAB : qb + 2 * SLAB]
                for ti, (W, X) in enumerate(((wq2h, uh), (wq2h, ul), (wq2l, uh))):
                    for hb in range(0, SLAB, 512):
                        he = min(hb + 512, SLAB)
                        mm = t.matmul(
                            ps_s[:, SS + hb : SS + he],
                            W[:, dd * 128 : (dd + 1) * 128], X[:, hb:he],
                            start=(dd == 0 and ti == 0),
                            stop=(dd == NDT - 1 and ti == 2),
                            skip_group_check=True,
                        )
                mm.then_inc(s_qp, 1)
            # ---- scores: 64 j-tiles, ordered by gather chunk so tiles of
            # chunk gi run as soon as collective gi has landed
            t.wait_ge(s_qs, it + 1)
            for pos in range(NT):
                tt = TORD[pos]
                gi = pos // (NT // NKG)
                if pos % (NT // NKG) == 0:
                    t.wait_ge(s_ccd, it * (NKG + 1) * C * 16 + (gi + 1) * C * 16)
                g = it * NT + pos
                if g >= 3:
                    t.wait_ge(s_ex, g - 2)     # slot free (exp 3 positions ago)
                so = (pos % 3) * SS
                kt = KS[:, tt * 128 : (tt + 1) * 128]
                for hb in range(0, SLAB, 512):
                    he = min(hb + 512, SLAB)
                    if not scores_lite:
                        t.matmul(
                            ps_s[:, so + hb : so + he], kt, Q1[:, hb:he],
                            start=True, stop=False, skip_group_check=True,
                        )
                    mm = t.matmul(
                        ps_s[:, so + hb : so + he], kt, Q2[:, hb:he],
                        start=scores_lite, stop=True, skip_group_check=True,
                    )
                mm.then_inc(s_sc, 1)
            # ---- attn: accumulate 64 tiles into ps_o
            t.wait_ge(s_oc, it)                # ps_o free
            for tt in range(1 if attn_lite else NT):
                t.wait_ge(s_vsc, it * NT + tt + 1)
                for hb in range(0, SLAB, 512):
                    he = min(hb + 512, SLAB)
                    mm = t.matmul(
                        ps_o[:, hb:he],
                        v_all[:, tt * VD : (tt + 1) * VD],
                        e_sb[:, tt * SLAB + hb : tt * SLAB + he],
                        start=(tt == 0), stop=(tt == (0 if attn_lite else NT - 1)),
                        skip_group_check=True,
                    )
            mm.then_inc(s_at, 1)

        # ---------------- VECTOR (DVE) ----------------
        ch_n = [0]

        @block.vector
        def _(v):
          def step(inst):
              inst.then_inc(s_ch, 1)
              ch_n[0] += 1

          def cw(v):
              v.wait_ge(s_ch, ch_n[0])

          for it in range(reps):
            # k split -> KSloc = [kh; kl]
            v.wait_ge(s_kp, it * NDT + NDT)
            if it > 0:
                v.wait_ge(s_gb, it * (NKG + 1) * 16)   # KSloc/vloc consumed
            step(v.tensor_copy(KSloc[0:64, :], ps_s[0:64, 0:SLAB]))
            step(v.tensor_copy(ktmp[64:128, :], ps_s[64:128, 0:SLAB]))
            cw(v)
            v.tensor_tensor(
                KSloc[64:128, :], ps_s[64:128, 0:SLAB], ktmp[64:128, :],
                op=ALU.subtract,
            ).then_inc(s_ks, 1)
            # v copy
            v.wait_ge(s_vp, it * NST + NST)
            v.tensor_copy(vloc[:, :], ps_v[:, :]).then_inc(s_vsp, 1)
            # q split -> Q1=[qh;ql], Q2=[ql;qh]
            v.wait_ge(s_qp, it * NDT + NDT)
            qsl = ps_s[:, SS : SS + SLAB]
            step(v.tensor_copy(Q1[0:64, :], qsl[0:64, :]))
            step(v.tensor_copy(Q2[64:128, :], qsl[64:128, :]))
            cw(v)
            v.tensor_tensor(
                Q1[64:128, :], qsl[64:128, :], Q2[64:128, :], op=ALU.subtract
            )
            v.tensor_tensor(
                Q2[0:64, :], qsl[0:64, :], Q1[0:64, :], op=ALU.subtract
            ).then_inc(s_qs, 1)
            # per-tile negated row-max: one fused pass over both psum
            # halves: scr = -max(lo, hi) elementwise, nb = min(scr)
            for pos in range(NT):
                tt = TORD[pos]
                v.wait_ge(s_sc, it * NT + pos + 1)
                if pos == 0 and it > 0:
                    v.wait_ge(s_g2, it * 16)   # stats_loc consumed
                    v.wait_ge(s_at, it)        # e_sb junk-write target free
                so = (pos % 3) * SS
                v.tensor_reduce(
                    stats_loc[:, tt : tt + 1],
                    ps_s[:, so : so + SLAB],
                    axis=AX, op=ALU.max, negate=True,
                ).then_inc(s_mx, 1)
            # stats math
            v.wait_ge(s_std, (it + 1) * 128)
            nb_v = stats_all[:, :].rearrange("p (c t) -> p t c", c=C, t=2 * NT)
            step(v.tensor_reduce(NM[:, :], nb_v[:, 0:NT, :], axis=AX, op=ALU.min))
            cw(v)
            for c in range(C):
                v.tensor_tensor(
                    darg[:, c * NT : (c + 1) * NT], NM[:, :],
                    stats_all[:, c * 2 * NT : c * 2 * NT + NT],
                    op=ALU.subtract,
                )
            v.tensor_tensor(
                wlarg[:, :], NM[:, :], stats_loc[:, 0:NT], op=ALU.subtract
            ).then_inc(s_sm, 1)
            v.wait_ge(s_sa, it + 1)
            dl_v = stats_all[:, :].rearrange("p (c t) -> p c t", c=C, t=2 * NT)
            step(v.tensor_tensor(
                wD[:, :].rearrange("p (c t) -> p c t", c=C, t=NT),
                w8[:, :].rearrange("p (c t) -> p c t", c=C, t=NT),
                dl_v[:, :, NT : 2 * NT],
                op=ALU.mult,
            ))
            cw(v)
            step(v.tensor_reduce(
                Dt[:, :],
                wD[:, :].rearrange("p (c t) -> p t c", c=C, t=NT),
                axis=AX, op=ALU.add,
            ))
            cw(v)
            step(v.reciprocal(Rt[:, :], Dt[:, :]))
            cw(v)
            step(v.tensor_tensor(f_sb[:, :], wl[:, :], Rt[:, :], op=ALU.mult))
            cw(v)
            # rescale v rows (in place)
            v.wait_ge(s_ccd, (it + 1) * (NKG + 1) * C * 16)
            for tt in range(NT):
                v.tensor_scalar_mul(
                    v_all[:, tt * VD : (tt + 1) * VD],
                    v_all[:, tt * VD : (tt + 1) * VD],
                    1.0 if noscale else f_sb[:, tt : tt + 1],
                ).then_inc(s_vsc, 1)

        # ---------------- SCALAR (ACT) ----------------
        @block.scalar
        def _(sc):
          for it in range(reps):
            # q arena stream on the ACT queue (serialized per-chunk levels)
            for dd in range(NDT):
                g = it * NDT + dd
                if g >= 2:
                    sc.wait_ge(s_qp, g - 1)
                sc.dma_start(
                    out=arena_q[:, (dd % 2) * 2 * SLAB : (dd % 2 + 1) * 2 * SLAB],
                    in_=qin_d[:, dd * 2 * SLAB : (dd + 1) * 2 * SLAB],
                ).then_inc(s_inq, 16)
                sc.wait_ge(s_inq, (it * NDT + dd + 1) * 16)
            for pos in range(NT):
                tt = TORD[pos]
                sc.wait_ge(s_mx, it * NT + pos + 1)
                if pos == 0 and it > 0:
                    sc.wait_ge(s_at, it)       # e_sb consumed by attn
                sc.activation(
                    e_sb[:, tt * SLAB : (tt + 1) * SLAB],
                    ps_s[:, (pos % 3) * SS : (pos % 3) * SS + SLAB],
                    ACTF.Exp,
                    bias=stats_loc[:, tt : tt + 1], scale=1.0,
                    accum_out=stats_loc[:, NT + tt : NT + tt + 1],
                ).then_inc(s_ex, 1)
            sc.wait_ge(s_sm, it + 1)
            sc.activation(w8[:, :], darg[:, :], ACTF.Exp)
            sc.activation(wl[:, :], wlarg[:, :], ACTF.Exp).then_inc(s_sa, 1)
            sc.wait_ge(s_at, it + 1)
            if it > 0:
                sc.wait_ge(s_out, it * 16)     # out_sb consumed
            sc.activation(out_sb[:, :], ps_o[:, :], ACTF.Copy).then_inc(s_oc, 1)

        # ---------------- GPSIMD: collectives ----------------
        @block.gpsimd
        def _(g):
          for it in range(reps):
            if it == 0:
                for wsb, wdr in ((wq2h, wq2h_d), (wq2l, wq2l_d),
                                 (wk2h, wk2h_d), (wk2l, wk2l_d), (wv, wv_d)):
                    g.dma_start(out=wsb[:, :], in_=wdr[:, :]).then_inc(s_w, 16)
            g.wait_ge(s_ks, it + 1)
            g.wait_ge(s_vsp, it + 1)
            if it > 0:
                g.wait_ge(s_cc1, it * (NKG + 1))   # cc1_in consumed
            for gi in range(NKG):
                g.dma_start(
                    out=cc1_ins[gi][:, :],
                    in_=KSloc[:, gi * KGW : (gi + 1) * KGW],
                ).then_inc(s_gb, 16)
            g.dma_start(out=ccv_in[:, :], in_=vloc[:, :]).then_inc(s_gb, 16)
            g.wait_ge(s_gb, it * (NKG + 1) * 16 + (NKG + 1) * 16)
            for gi in range(NKG):
                if no_cc:
                    g.nop().then_inc(s_cc1, 1)
                else:
                    g.collective_compute(
                        "AllGather", mybir.AluOpType.bypass,
                        replica_groups=[list(range(C))],
                        ins=[cc1_ins[gi][:, :].opt()],
                        outs=[cc1_outs[gi][:, :].opt()],
                    ).then_inc(s_cc1, 1)
                g.wait_ge(s_cc1, it * (NKG + 1) + gi + 1)
                g.wait_ge(s_ccd, it * (NKG + 1) * C * 16 + gi * C * 16)
                for c in range(C):
                    g.dma_start(
                        out=KS[:, c * SLAB + gi * KGW : c * SLAB + (gi + 1) * KGW],
                        in_=cc1_outs[gi][c * 128 : (c + 1) * 128, :],
                    ).then_inc(s_ccd, 16)
            if no_cc:
                g.nop().then_inc(s_cc1, 1)
            else:
                g.collective_compute(
                    "AllGather", mybir.AluOpType.bypass,
                    replica_groups=[list(range(C))],
                    ins=[ccv_in[:, :].opt()],
                    outs=[ccv_out[:, :].opt()],
                ).then_inc(s_cc1, 1)
            g.wait_ge(s_cc1, it * (NKG + 1) + NKG + 1)
            g.wait_ge(s_ccd, it * (NKG + 1) * C * 16 + NKG * C * 16)
            for c in range(C):
                g.dma_start(
                    out=v_all[:, c * (SLAB // 2) : (c + 1) * (SLAB // 2)],
                    in_=ccv_out[c * 128 : (c + 1) * 128, :],
                ).then_inc(s_ccd, 16)
            # stats gather
            g.wait_ge(s_ex, (it + 1) * NT)
            if it > 0:
                g.wait_ge(s_cc2, it)
            g.dma_start(out=cc2_in[:, :], in_=stats_loc[:, :]).then_inc(s_g2, 16)
            g.wait_ge(s_g2, (it + 1) * 16)
            if no_cc:
                g.nop().then_inc(s_cc2, 1)
            else:
                g.collective_compute(
                    "AllGather", mybir.AluOpType.bypass,
                    replica_groups=[list(range(C))],
                    ins=[cc2_in[:, :].opt()],
                    outs=[cc2_out[:, :].opt()],
                ).then_inc(s_cc2, 1)
            g.wait_ge(s_cc2, it + 1)
            for c in range(C):
                g.dma_start(
                    out=stats_all[:, c * 2 * NT : (c + 1) * 2 * NT],
                    in_=cc2_out[c * 128 : (c + 1) * 128, :],
                ).then_inc(s_std, 16)

    nc.finalize()
    return nc


# ------------------------- host side -------------------------

def _split_bf16(x):
    import ml_dtypes

    hi = x.astype(ml_dtypes.bfloat16)
    lo = (x - hi.astype(np.float32)).astype(ml_dtypes.bfloat16)
    return hi, lo


def _tile_cols(xT, w):
    """[d, s] -> [128, (d/128)*w] with col = dd*w + i (s == w per d-tile)."""
    dd = xT.shape[0] // 128
    return np.ascontiguousarray(
        xT.reshape(dd, 128, w).transpose(1, 0, 2).reshape(128, dd * w)
    )


def build_in_maps2(inputs, seq=8192, d=1024):
    import ml_dtypes

    bf = ml_dtypes.bfloat16
    SLAB = seq // C
    NDT = d // 128

    qw = (inputs["query_weights"] / np.sqrt(np.float32(QK))).astype(np.float32)
    wqh, wql = _split_bf16(qw)
    wkh, wkl = _split_bf16(inputs["key_weights"].astype(np.float32))

    def dup_tile(w):
        w2 = np.concatenate([w.astype(np.float32)] * 2, axis=1)  # [d, 128]
        return _tile_cols(w2, 128).astype(bf)

    w_maps = {
        "wq2h": dup_tile(wqh), "wq2l": dup_tile(wql),
        "wk2h": dup_tile(wkh), "wk2l": dup_tile(wkl),
        "wv": _tile_cols(
            inputs["value_weights"].astype(np.float32), VD
        ).astype(bf),
    }

    def slab_hi_lo(xT):
        """[d, SLAB] -> [128, NDT*2*SLAB] with col = dd*2*SLAB + h*SLAB + i."""
        hi, lo = _split_bf16(xT)
        ht = _tile_cols(hi.astype(np.float32), SLAB).reshape(128, NDT, SLAB)
        lt = _tile_cols(lo.astype(np.float32), SLAB).reshape(128, NDT, SLAB)
        return np.ascontiguousarray(
            np.concatenate([ht[:, :, None, :], lt[:, :, None, :]], axis=2)
            .reshape(128, NDT * 2 * SLAB)
        ).astype(bf)

    def _vin_tile(xT):
        """[d, SLAB] -> [128, NST*NDT*128], col = st*NDT*128 + dd*128 + jj."""
        t = _tile_cols(xT, SLAB).reshape(128, NDT, SLAB // 128, 128)
        return np.ascontiguousarray(
            t.transpose(0, 2, 1, 3).reshape(128, -1)
        )

    qT = np.ascontiguousarray(inputs["queries"].T).astype(np.float32)
    kT = np.ascontiguousarray(inputs["keys"].T).astype(np.float32)
    vT = np.ascontiguousarray(inputs["values"].T).astype(np.float32)

    in_maps = []
    for c in range(C):
        sl = slice(c * SLAB, (c + 1) * SLAB)
        m = {
            "qin": slab_hi_lo(qT[:, sl]),
            "kin": slab_hi_lo(kT[:, sl]),
            "vin": _vin_tile(vT[:, sl]).astype(bf),
        }
        m.update(w_maps)
        in_maps.append(m)
    return in_maps


def assemble_out2(results, seq=8192):
    SLAB = seq // C
    full = np.zeros((seq, VD), np.float32)
    for c in range(C):
        o = np.asarray(results[c]["out"], dtype=np.float32)  # [VD, SLAB]
        full[c * SLAB : (c + 1) * SLAB] = o.T
    return full


def run_spmd_staged(nc, in_maps, profile_dir=None):
    """run_bass_via_pjrt with inputs pre-staged on-device (blocks until all
    shards are resident) so the 8 cores launch aligned instead of staggered
    by per-device input-transfer time."""
    import jax
    import numpy as np_
    from jax.sharding import Mesh, PartitionSpec, NamedSharding
    from jax.experimental.shard_map import shard_map
    import concourse.mybir as mybir
    from concourse import bass2jax

    bass2jax.install_neuronx_cc_hook()
    n_cores = len(in_maps)

    partition_name = (
        nc.partition_id_tensor.name if nc.partition_id_tensor else None
    )
    in_names, out_names, out_avals, zero_outs = [], [], [], []
    for alloc in nc.m.functions[0].allocations:
        if not isinstance(alloc, mybir.MemoryLocationSet):
            continue
        name = alloc.memorylocations[0].name
        if alloc.kind == "ExternalInput":
            if name != partition_name:
                in_names.append(name)
        elif alloc.kind == "ExternalOutput":
            out_names.append(name)
            shape = tuple(alloc.tensor_shape)
            dtype = mybir.dt.np(alloc.dtype)
            out_avals.append(jax.core.ShapedArray(shape, dtype))
            zero_outs.append(np_.zeros(shape, dtype))
    n_params = len(in_names)
    n_outs = len(out_avals)
    all_names = in_names + out_names
    if partition_name is not None:
        all_names = all_names + [partition_name]

    def _body(*args):
        operands = list(args)
        if partition_name is not None:
            operands.append(bass2jax.partition_id_tensor())
        outs = bass2jax._bass_exec_p.bind(
            *operands,
            out_avals=tuple(out_avals),
            in_names=tuple(all_names),
            out_names=tuple(out_names),
            lowering_input_output_aliases=(),
            sim_require_finite=True,
            sim_require_nnan=True,
            nc=nc,
        )
        return tuple(outs)

    devices = jax.devices()[:n_cores]
    mesh = Mesh(np_.asarray(devices), ("core",))
    spec = NamedSharding(mesh, PartitionSpec("core"))
    sharded = jax.jit(
        shard_map(
            _body,
            mesh=mesh,
            in_specs=(PartitionSpec("core"),) * (n_params + n_outs),
            out_specs=(PartitionSpec("core"),) * n_outs,
            check_rep=False,
        ),
        keep_unused=True,
    )
    concat_in = [
        np_.concatenate([np_.asarray(in_maps[c][nm]) for c in range(n_cores)], axis=0)
        for nm in in_names
    ]
    concat_zero = [
        np_.zeros((n_cores * z.shape[0], *z.shape[1:]), z.dtype) for z in zero_outs
    ]
    staged = [jax.device_put(a, spec) for a in concat_in + concat_zero]
    jax.block_until_ready(staged)

    if profile_dir is not None:
        from antenv.axon_hooks import get_axon_ntff_profile_hook

        hook = get_axon_ntff_profile_hook()
        with hook(profile_dir, list(range(n_cores))):
            out_arrs = sharded(*staged)
            jax.block_until_ready(out_arrs)
    else:
        out_arrs = sharded(*staged)
    return [
        {
            nm: np_.asarray(out_arrs[i]).reshape(n_cores, *out_avals[i].shape)[c]
            for i, nm in enumerate(out_names)
        }
        for c in range(n_cores)
    ]


def kernel(queries, keys, values, query_weights, key_weights, value_weights):
    import sys

    for p in ("/opt/trn_rl_repo",):
        if p not in sys.path:
            sys.path.insert(0, p)

    seq, d = queries.shape
    inputs = {
        "queries": queries, "keys": keys, "values": values,
        "query_weights": query_weights, "key_weights": key_weights,
        "value_weights": value_weights,
    }
    in_maps = build_in_maps2(inputs, seq=seq, d=d)
    nc = build_nc2(seq=seq, d=d)
    results = run_spmd_staged(nc, in_maps)
    return assemble_out2(results, seq=seq)
